# revision 1
# baseline (speedup 1.0000x reference)
"""Trainium2 Bass kernel for multi-head self-attention (B=4, N=2048, C=1024, H=16).

Sharding: 8 cores = 4 batches x 2 head-groups (8 heads each). Per core:
  - V rows and Q^T/K^T packs from x[b] (PE), V interleaved with a per-head
    ones column so PV also produces softmax row sums
  - flash-style attention per (head, 1024-q chunk): S^T tiles -> exp ->
    PV with out layout [q, d+1] (full 128 output partitions)
  - normalize by 1/rowsum via per-partition tensor_scalar_mul, PE-transpose
    back to [d, q] for the output projection
Host preps per-core inputs and sums the two partial projections per batch.
"""

import numpy as np
import ml_dtypes

import concourse.bass as bass
import concourse.mybir as mybir
import concourse.tile as tile
from concourse import bacc
from concourse.ap import AP
from concourse.bass_utils import run_bass_kernel_spmd

BF16 = mybir.dt.bfloat16
F32 = mybir.dt.float32
FP8 = mybir.dt.float8e4
Exp = mybir.ActivationFunctionType.Exp
DR = mybir.MatmulPerfMode.DoubleRow
bf = ml_dtypes.bfloat16

B, N, C = 4, 2048, 1024
H, D = 16, 64
N_CORES = 8
HPC = H // 2     # heads per core (8)
PAIRS = HPC // 2  # head pairs per core (4)
CT = C // 128    # contraction tiles over C (8)
KT = N // 128    # key tiles (16)
RT = N // 128    # row tiles for V (16)
QT4 = N // 512   # 512-wide q chunks (4)
QC = 2           # 1024-wide q chunks
SCALE = 1.0 / float(np.sqrt(D))

import os
USE_FP8 = os.environ.get("K_FP8", "0") == "1"

_COMPILED = {}


def _build(with_bias: bool):
    nc = bacc.Bacc("TRN2", target_bir_lowering=False, debug=False,
                   num_devices=N_CORES)
    xt_d = nc.dram_tensor("xt", [C, N], BF16, kind="ExternalInput").ap()
    wqk_d = nc.dram_tensor("wqk", [C, 1024], BF16, kind="ExternalInput").ap()
    wv_d = nc.dram_tensor("wv", [C, 512], BF16, kind="ExternalInput").ap()
    wpr_d = nc.dram_tensor("wpr", [512, C], BF16, kind="ExternalInput").ap()
    id_d = nc.dram_tensor("ident", [128, 128], BF16, kind="ExternalInput").ap()
    if with_bias:
        bqk_d = nc.dram_tensor("bqk", [1, 1024], BF16, kind="ExternalInput").ap()
        bv_d = nc.dram_tensor("bv", [1, 512], BF16, kind="ExternalInput").ap()
        bpr_d = nc.dram_tensor("bpr", [1, C], BF16, kind="ExternalInput").ap()
    out_d = nc.dram_tensor("out", [N, C], BF16, kind="ExternalOutput").ap()

    with tile.TileContext(nc) as tc:
        with (
            tc.tile_pool(name="persist", bufs=1) as pp,
            tc.tile_pool(name="pt", bufs=24) as pt_pool,
            tc.tile_pool(name="st", bufs=3) as st_pool,
            tc.tile_pool(name="rc", bufs=6) as rc_pool,
            tc.tile_pool(name="q8s", bufs=4) as q8s_pool,
            tc.tile_pool(name="oc", bufs=4) as oc_pool,
            tc.tile_pool(name="ps_sc", bufs=2, space="PSUM") as ps_sc,
            tc.tile_pool(name="ps_pv", bufs=1, space="PSUM") as ps_pv,
            tc.tile_pool(name="ps_aux", bufs=2, space="PSUM") as ps_aux,
        ):
            # ---------------- input loads ----------------
            # alternate between the SP/HWDGE and Pool/SWDGE descriptor-gen
            # pipelines so bursts of DMAs generate in parallel
            dma_rr = [0]

            def dma(dst, srcap):
                # HWDGE gen is ~625ns/DMA vs SWDGE ~1040ns: weight 2:1
                eng = (nc.sync, nc.sync, nc.gpsimd)[dma_rr[0] % 3]
                dma_rr[0] += 1
                eng.dma_start(dst, srcap)

            xt_sb = [pp.tile([128, N], BF16, tag=f"xt{ct}", name=f"xt{ct}")
                     for ct in range(CT)]
            wv_sb = [pp.tile([128, 512], BF16, tag=f"wv{ct}", name=f"wv{ct}")
                     for ct in range(CT)]
            wqk_sb = [pp.tile([128, 1024], BF16, tag=f"wqk{ct}", name=f"wqk{ct}")
                      for ct in range(CT)]
            for ct in range(CT):
                dma(wv_sb[ct][:], wv_d[ct * 128:(ct + 1) * 128, :])
                dma(xt_sb[ct][:, 0:512], xt_d[ct * 128:(ct + 1) * 128, 0:512])
            for ct in range(CT):
                dma(wqk_sb[ct][:], wqk_d[ct * 128:(ct + 1) * 128, :])
            for q4 in range(1, QT4):
                qsl = slice(q4 * 512, (q4 + 1) * 512)
                for ct in range(CT):
                    dma(xt_sb[ct][:, qsl], xt_d[ct * 128:(ct + 1) * 128, qsl])
            wpr_sb = []
            for cp in range(PAIRS):
                t = pp.tile([128, C], BF16, tag=f"wpr{cp}", name=f"wpr{cp}")
                dma(t[:], wpr_d[cp * 128:(cp + 1) * 128, :])
                wpr_sb.append(t)
            ident = pp.tile([128, 128], BF16, tag="ident")
            dma(ident[:], id_d[:])

            def xt_s(ct, sl):
                return xt_sb[ct][:, sl]

            def wqk_s(ct, sl):
                return wqk_sb[ct][:, sl]

            def wv_s(ct):
                return wv_sb[ct][:]

            def wpr_s(cp, sl):
                return wpr_sb[cp][:, sl]
            if with_bias:
                ones = pp.tile([1, N], BF16, tag="ones")
                nc.vector.memset(ones[:], 1.0)
                bqk_sb = pp.tile([1, 1024], BF16, tag="bqk")
                dma(bqk_sb[:], bqk_d[:])
                bv_sb = pp.tile([1, 512], BF16, tag="bv")
                dma(bv_sb[:], bv_d[:])
                bpr_sb = pp.tile([1, C], BF16, tag="bpr")
                dma(bpr_sb[:], bpr_d[:])

            # ---------------- persistent working tiles ----------------
            if USE_FP8:
                # fp8 Q/K packs: tile p holds the pair's 2 heads at partition
                # bases 0/64 (base 96 is not addressable by matmul operands);
                # per head layout [32, 2(d-half), N]
                q8_sb = [pp.tile([128, 2 * N], FP8, tag=f"q8{p}", name=f"q8{p}")
                         for p in range(PAIRS)]
                k8_sb = [pp.tile([128, 2 * N], FP8, tag=f"k8{p}", name=f"k8{p}")
                         for p in range(PAIRS)]
            else:
                qt_sb = [pp.tile([128, N], BF16, tag=f"qt{p}", name=f"qt{p}")
                         for p in range(PAIRS)]
                kt_sb = [pp.tile([128, N], BF16, tag=f"kt{p}", name=f"kt{p}")
                         for p in range(PAIRS)]
            va_sb = [pp.tile([128, HPC * 65], BF16, tag=f"va{rt}", name=f"va{rt}")
                     for rt in range(RT)]
            plhsT = [pp.tile([128, N], BF16, tag=f"pl{p}", name=f"pl{p}")
                     for p in range(PAIRS)]

            # ---- V row tile: out[r, h*65+d] = sum_c xT[c, r] * Wv[c, h*64+d]
            v_acc = {}

            def v_tile_half(rt, half):
                va3 = va_sb[rt][:].rearrange("p (h d) -> p h d", d=65)
                rsl = slice(rt * 128, (rt + 1) * 128)
                if half == 0:
                    nc.vector.memset(va3[:, :, 64:65], 1.0)
                    v_acc[rt] = ps_aux.tile([128, 512], F32, tag="aux",
                                            name="acc_v")
                acc = v_acc[rt]
                for ct in range(4 * half, 4 * half + 4):
                    nc.tensor.matmul(acc[:], xt_s(ct, rsl), wv_s(ct),
                                     start=(ct == 0),
                                     stop=(not with_bias and ct == CT - 1))
                if half == 0:
                    return
                if with_bias:
                    nc.tensor.matmul(acc[:], ones[0:1, 0:128], bv_sb[:],
                                     start=False, stop=True)
                src3 = acc[:].rearrange("p (h d) -> p h d", d=64)
                nc.vector.tensor_copy(va3[:, :, 0:64], src3)
                del v_acc[rt]

            def v_tile(rt):
                v_tile_half(rt, 0)
                v_tile_half(rt, 1)

            # ---- Q^T/K^T pack chunk: out[o, q] = sum_c W[c, o] * xT[c, q]
            # split into two PE half-units so fillers stay fine-grained
            qk_acc = {}

            def qk_chunk_half(p, i, half):
                qk, q4 = i // QT4, i % QT4
                osl = slice(qk * 512 + p * 128, qk * 512 + (p + 1) * 128)
                qsl = slice(q4 * 512, (q4 + 1) * 512)
                if half == 0:
                    qk_acc[(p, i)] = ps_aux.tile([128, 512], F32, tag="aux",
                                                 name="acc_qk")
                acc = qk_acc[(p, i)]
                for ct in range(4 * half, 4 * half + 4):
                    nc.tensor.matmul(acc[:], wqk_s(ct, osl),
                                     xt_s(ct, qsl), start=(ct == 0),
                                     stop=(not with_bias and ct == CT - 1))
                if half == 0:
                    return
                if with_bias:
                    nc.tensor.matmul(acc[:], bqk_sb[0:1, osl], ones[0:1, qsl],
                                     start=False, stop=True)
                if USE_FP8:
                    stage = q8s_pool.tile([128, 512], FP8, tag="q8s",
                                          name="stage8")
                    nc.vector.tensor_copy(stage[:], acc[:])
                    dst = (q8_sb, k8_sb)[qk]
                    for hl in range(2):
                        for ih in range(2):
                            dma(dst[p][64 * hl:64 * hl + 32,
                                       ih * N + q4 * 512:ih * N + (q4 + 1) * 512],
                                stage[64 * hl + 32 * ih:64 * hl + 32 * ih + 32, :])
                else:
                    dst = (qt_sb, kt_sb)[qk]
                    nc.vector.tensor_copy(dst[p][:, qsl], acc[:])
                del qk_acc[(p, i)]

            def qk_chunk(p, i):
                qk_chunk_half(p, i, 0)
                qk_chunk_half(p, i, 1)

            # two chunks with ct-interleaved matmuls: both chains advance as
            # each wqk tile lands (avoids head-of-line blocking on the
            # streaming weight DMAs)
            def qk_chunk2(p, ia, ib):
                accs = {}
                for i in (ia, ib):
                    accs[i] = ps_aux.tile([128, 512], F32, tag="aux",
                                          name="acc_qk")
                for ct in range(CT):
                    for i in (ia, ib):
                        qk, q4 = i // QT4, i % QT4
                        osl = slice(qk * 512 + p * 128,
                                    qk * 512 + (p + 1) * 128)
                        qsl = slice(q4 * 512, (q4 + 1) * 512)
                        nc.tensor.matmul(accs[i][:], wqk_s(ct, osl),
                                         xt_s(ct, qsl), start=(ct == 0),
                                         stop=(not with_bias and ct == CT - 1))
                for i in (ia, ib):
                    qk, q4 = i // QT4, i % QT4
                    osl = slice(qk * 512 + p * 128, qk * 512 + (p + 1) * 128)
                    qsl = slice(q4 * 512, (q4 + 1) * 512)
                    if with_bias:
                        nc.tensor.matmul(accs[i][:], bqk_sb[0:1, osl],
                                         ones[0:1, qsl], start=False, stop=True)
                    dst = (qt_sb, kt_sb)[qk]
                    nc.vector.tensor_copy(dst[p][:, qsl], accs[i][:])

            # ---- partial output projection for one 128-row q tile (half) ----
            def proj_half(qt_i, nch):
                qsl = slice(qt_i * 128, (qt_i + 1) * 128)
                nsl = slice(nch * 512, (nch + 1) * 512)
                if nch == 0:
                    proj_oc[qt_i] = oc_pool.tile([128, 1024], BF16, tag="oc",
                                                 name="oc")
                oc = proj_oc[qt_i]
                acc = ps_aux.tile([128, 512], F32, tag="aux", name="acc_pr")
                for cp in range(PAIRS):
                    nc.tensor.matmul(acc[:], plhsT[cp][:, qsl],
                                     wpr_s(cp, nsl), start=(cp == 0),
                                     stop=(not with_bias and cp == PAIRS - 1))
                if with_bias:
                    nc.tensor.matmul(acc[:], ones[0:1, 0:128], bpr_sb[0:1, nsl],
                                     start=False, stop=True)
                if qt_i >= 14:
                    # tail-critical: put one half-copy on the idle ACT engine
                    # and fan the store over both DGE paths
                    (nc.vector.tensor_copy if nch == 0
                     else nc.scalar.copy)(oc[:, nsl], acc[:])
                    if nch == 1:
                        nc.sync.dma_start(out_d[qsl, 0:512], oc[:, 0:512])
                        nc.gpsimd.dma_start(out_d[qsl, 512:1024],
                                            oc[:, 512:1024])
                else:
                    nc.vector.tensor_copy(oc[:, nsl], acc[:])
                    if nch == 1:
                        dma(out_d[qsl, :], oc[:])

            proj_oc = {}

            # ---- attention segment: one head, one 1024-wide q chunk ----
            # pv accum regions: q-tile t<7 at col 65*t (bank 0), t=7 at col
            # 512 (bank 1). start=True on the first matmul into each bank
            # marks the whole bank pending-zero, so every region's first
            # write lands fresh; stop=True only on the bank's last matmul.
            pending_pe = []
            # decoupled PV stream: (exp_idx, closure). The softmax stream
            # (scores+exp) runs ahead; PV/drain/transpose work is pumped into
            # PE's spare capacity, bounded by the pt pool depth.
            pv_queue = []
            exp_idx = [0]
            PT_LAG = 22

            def pump_pv(n):
                for _ in range(n):
                    if pv_queue:
                        pv_queue.pop(0)[1]()

            def segment(h, qc, fillers, budget=1, inline_tr=False):
                p, hl = h // 2, h % 2
                g, r = h // 4, h % 4
                pv = ps_pv.tile([128, 1024], F32, tag="pv", name="pv")
                if hl == 0:
                    st_seg[p] = st_pool.tile([128, 1024], BF16, tag="st",
                                             name="st")
                st = st_seg[p]

                def pv_region(t):
                    c0 = 65 * t if t < 7 else 512
                    return pv[:, c0:c0 + 65]

                def scores(kt):
                    sc = ps_sc.tile([128, 1024], F32, tag="sc", name="sc")
                    ksl = slice(kt * 128, (kt + 1) * 128)
                    for qh in range(2):
                        q0 = qc * 1024 + qh * 512
                        if USE_FP8:
                            l3 = k8_sb[p][64 * hl:64 * hl + 32, :].rearrange(
                                "p (i n) -> p i n", i=2)[:, :, ksl]
                            r3 = q8_sb[p][64 * hl:64 * hl + 32, :].rearrange(
                                "p (i n) -> p i n", i=2)[:, :, q0:q0 + 512]
                            nc.tensor.matmul(
                                sc[:, qh * 512:(qh + 1) * 512], l3, r3,
                                start=True, stop=True, perf_mode=DR)
                        else:
                            nc.tensor.matmul(
                                sc[:, qh * 512:(qh + 1) * 512],
                                kt_sb[p][64 * hl:64 * hl + 64, ksl],
                                qt_sb[p][64 * hl:64 * hl + 64, q0:q0 + 512],
                                start=True, stop=True)
                    return sc

                def pv_group(kt, pt):
                    for t in range(8):
                        nc.tensor.matmul(
                            pv_region(t), pt[:, 128 * t:128 * (t + 1)],
                            va_sb[kt][:, h * 65:(h + 1) * 65],
                            start=(kt == 0 and t in (0, 7)),
                            stop=(kt == KT - 1),
                            skip_group_check=True)

                scs = [scores(0), scores(1)]
                for kt in range(KT):
                    # pt-slot WAR: the PV group reading the tile this exp's
                    # slot will recycle must be emitted before the exp
                    while pv_queue and (pv_queue[0][0] is None or
                                        pv_queue[0][0] <= exp_idx[0] - PT_LAG):
                        pump_pv(1)
                    pt = pt_pool.tile([128, 1024], BF16, tag="pt", name="pt")
                    nc.scalar.activation(pt[:], scs[kt % 2][:], Exp, scale=SCALE)
                    # scores lookahead first: the exp stream advances as soon
                    # as PE delivers them, ahead of slower filler units
                    if kt + 2 < KT:
                        scs[kt % 2] = scores(kt + 2)
                    # early kt slots go to this segment's deadline-critical
                    # fillers (self-hosted K chunks); deferred transposes of
                    # the previous pair run mid-segment
                    if kt < 6:
                        if fillers:
                            for _ in range(budget):
                                if fillers:
                                    fillers.pop(0)()
                        elif pending_pe:
                            pending_pe.pop(0)()
                        else:
                            pump_pv(2)
                    else:
                        if pending_pe:
                            pending_pe.pop(0)()
                        elif fillers:
                            for _ in range(budget):
                                if fillers:
                                    fillers.pop(0)()
                        else:
                            pump_pv(2)
                    pv_queue.append((exp_idx[0], lambda kt=kt, pt=pt:
                                     pv_group(kt, pt)))
                    exp_idx[0] += 1

                # normalize: recip of row sums, per-partition scalar multiply
                def drain(pv=pv, st=st, hl=hl):
                    recip = rc_pool.tile([128, 8], F32, tag="rc", name="recip")
                    nc.vector.reciprocal(recip[:, 0:7], pv[:, 64:455:65])
                    st3 = st[:].rearrange("p (t d) -> p t d", d=128)[
                        :, 0:7, 64 * hl:64 * hl + 64]
                    pv3 = pv[:, 0:455].rearrange("p (t d) -> p t d", d=65)[
                        :, :, 0:64]
                    rc3 = recip[:, 0:7].rearrange("p (t d) -> p t d", d=1)
                    nc.vector.tensor_mul(st3, pv3,
                                         rc3.broadcast_to((128, 7, 64)))
                    nc.vector.reciprocal(recip[:, 7:8], pv[:, 576:577])
                    nc.vector.tensor_scalar_mul(
                        st[:, 128 * 7 + 64 * hl:128 * 7 + 64 * hl + 64],
                        pv[:, 512:576], recip[:, 7:8])

                pv_queue.append((None, drain))
                if hl == 1:
                    # transpose [q, d-pair] -> [d-pair, q] via PE, then one
                    # copy into the proj lhsT pack; deferred into the next
                    # segment's filler slots so the PE stream never stalls
                    # on the DVE normalize chain
                    tr_box = {}

                    def tr_half(lo, st=st, box=tr_box):
                        if lo == 0:
                            box["aux"] = ps_aux.tile([128, 512], F32,
                                                     tag="aux", name="tr")
                        tr = box["aux"].bitcast(BF16)
                        for t in range(lo, lo + 4):
                            nc.tensor.transpose(tr[:, 128 * t:128 * (t + 1)],
                                                st[:, 128 * t:128 * (t + 1)],
                                                ident[:])

                    def tr_copy(p=p, qc=qc, box=tr_box):
                        nc.vector.tensor_copy(
                            plhsT[p][:, qc * 1024:(qc + 1) * 1024],
                            box["aux"].bitcast(BF16))

                    pv_queue.append((None, lambda: tr_half(0)))
                    pv_queue.append((None, lambda: tr_half(4)))
                    pv_queue.append((None, tr_copy))
                if inline_tr:
                    pump_pv(len(pv_queue))

            st_seg = {}

            # ---- 512-wide half segment: used to split the very last
            # (head, q-chunk) so the final projection tiles overlap the
            # second half's softmax stream
            def half_segment(h, qc, qh, fillers):
                p, hl = h // 2, h % 2
                pv = ps_pv.tile([128, 1024], F32, tag="pv", name="pv")
                if hl == 0 and qh == 0:
                    st_seg[p] = st_pool.tile([128, 1024], BF16, tag="st",
                                             name="st")
                st = st_seg[p]
                q0 = qc * 1024 + qh * 512

                def scores(kt):
                    sc = ps_sc.tile([128, 1024], F32, tag="sc", name="sc")
                    ksl = slice(kt * 128, (kt + 1) * 128)
                    nc.tensor.matmul(
                        sc[:, 0:512],
                        kt_sb[p][64 * hl:64 * hl + 64, ksl],
                        qt_sb[p][64 * hl:64 * hl + 64, q0:q0 + 512],
                        start=True, stop=True)
                    return sc

                scs = [scores(0), scores(1)]
                for kt in range(KT):
                    pt = pt_pool.tile([128, 1024], BF16, tag="pt", name="pt")
                    nc.scalar.activation(pt[:, 0:512], scs[kt % 2][:, 0:512],
                                         Exp, scale=SCALE)
                    if fillers:
                        fillers.pop(0)()
                    if kt + 2 < KT:
                        scs[kt % 2] = scores(kt + 2)
                    for t in range(4):
                        nc.tensor.matmul(
                            pv[:, 65 * t:65 * t + 65],
                            pt[:, 128 * t:128 * (t + 1)],
                            va_sb[kt][:, h * 65:(h + 1) * 65],
                            start=(kt == 0 and t == 0),
                            stop=(kt == KT - 1),
                            skip_group_check=True)

                recip = rc_pool.tile([128, 8], F32, tag="rc", name="recip")
                nc.vector.reciprocal(recip[:, 0:4], pv[:, 64:260:65])
                st3 = st[:].rearrange("p (t d) -> p t d", d=128)[
                    :, 4 * qh:4 * qh + 4, 64 * hl:64 * hl + 64]
                pv3 = pv[:, 0:260].rearrange("p (t d) -> p t d", d=65)[
                    :, :, 0:64]
                rc3 = recip[:, 0:4].rearrange("p (t d) -> p t d", d=1)
                nc.vector.tensor_mul(st3, pv3, rc3.broadcast_to((128, 4, 64)))
                if hl == 1:
                    aux = ps_aux.tile([128, 512], F32, tag="aux", name="tr")
                    tr = aux.bitcast(BF16)
                    for t in range(4):
                        nc.tensor.transpose(
                            tr[:, 128 * t:128 * (t + 1)],
                            st[:, 128 * (4 * qh + t):128 * (4 * qh + t + 1)],
                            ident[:])
                    nc.vector.tensor_copy(
                        plhsT[p][:, q0:q0 + 512], tr[:, 0:512])

            # ---------------- emission schedule ----------------
            # head: only what the first exps need (V rows 0-1, pair 0's
            # low-q Q packs and first K chunk); everything else fills
            # segment slots just-in-time
            v_tile(0)
            v_tile(1)
            v_tile(2)
            if USE_FP8:
                for i in (0, 1, 4):
                    qk_chunk(0, i)
            else:
                qk_chunk2(0, 0, 1)
                qk_chunk(0, 4)

            def pack_units(p, chunks):
                return [lambda p=p, i=i, half=half: qk_chunk_half(p, i, half)
                        for i in chunks for half in range(2)]

            def v_units(rts):
                return [lambda rt=rt, half=half: v_tile_half(rt, half)
                        for rt in rts for half in range(2)]

            # per-segment fillers, sized ~0.5-1 exp each, due before use:
            # seg 0 runs at 2 pops/kt to finish the V rows just-in-time;
            # pair p's K + low-q Q packs before seg 2p; high-q Q (needed by
            # qc1 only) in segs 5-8; projection of qc0 rows spread over qc1
            seg_fill = {i: [] for i in range(2 * HPC)}
            seg_fill[0] = (v_units([3]) + pack_units(0, [5])
                           + v_units([4, 5, 6]) + pack_units(0, [6])
                           + v_units([7, 8, 9]) + pack_units(0, [7])
                           + v_units([10, 11, 12, 13, 14, 15]))
            seg_fill[1] = pack_units(1, [0, 1, 4])
            seg_fill[2] = pack_units(1, [5, 6, 7]) + pack_units(2, [0])
            seg_fill[3] = pack_units(2, [1, 4]) + pack_units(3, [0])
            seg_fill[4] = pack_units(2, [5, 6, 7]) + pack_units(3, [1])
            seg_fill[5] = pack_units(3, [4]) + pack_units(0, [2, 3])
            seg_fill[6] = pack_units(3, [5, 6, 7])
            seg_fill[7] = pack_units(1, [2, 3])
            seg_fill[8] = pack_units(2, [2, 3])
            seg_fill[9] = pack_units(3, [2, 3])
            for s in range(HPC):
                seg_fill[8 + s] += [
                    lambda qt_i=s, nch=j: proj_half(qt_i, nch)
                    for j in range(2)]

            carry = []
            for qc in range(QC):
                for h in range(HPC):
                    if qc == 1 and h == HPC - 1:
                        break
                    fl = carry + seg_fill[qc * HPC + h]
                    segment(h, qc, fl,
                            budget=2 if (qc == 0 and h == 0) else 1,
                            inline_tr=(qc == 0 and h == HPC - 1))
                    carry = fl
            pump_pv(len(pv_queue))
            half_segment(HPC - 1, 1, 0, carry + seg_fill[15])
            tail_fill = [lambda qt_i=8 + (j // 2), nch=j % 2:
                         proj_half(qt_i, nch) for j in range(8)]
            half_segment(HPC - 1, 1, 1, tail_fill)
            for w in pending_pe:
                w()
            pending_pe.clear()
            for qt_i in range(12, 16):
                proj_half(qt_i, 0)
                proj_half(qt_i, 1)

    nc.compile()
    return nc


def _get_nc(with_bias=False):
    if with_bias not in _COMPILED:
        _COMPILED[with_bias] = _build(with_bias)
    return _COMPILED[with_bias]


def _prep_in_maps(x, W_qkv, b_qkv, W_proj, b_proj, with_bias):
    ident = np.eye(128).astype(bf)
    in_maps = []
    for c in range(N_CORES):
        b = c // 2
        g = c % 2
        hs = slice(g * 512, (g + 1) * 512)
        xt = np.ascontiguousarray(x[b].T).astype(bf)
        wq = W_qkv[:, 0:C][:, hs]
        wk = W_qkv[:, C:2 * C][:, hs]
        wv = W_qkv[:, 2 * C:3 * C][:, hs]
        wqk = np.ascontiguousarray(np.concatenate([wq, wk], axis=1)).astype(bf)
        wpr = np.ascontiguousarray(W_proj[hs, :]).astype(bf)
        m = {
            "xt": xt, "wqk": wqk, "wv": np.ascontiguousarray(wv).astype(bf),
            "wpr": wpr, "ident": ident,
        }
        if with_bias:
            bq = b_qkv[0:C][hs]
            bk = b_qkv[C:2 * C][hs]
            bvv = b_qkv[2 * C:3 * C][hs]
            m["bqk"] = np.concatenate([bq, bk])[None, :].astype(bf)
            m["bv"] = np.ascontiguousarray(bvv[None, :]).astype(bf)
            m["bpr"] = ((b_proj if g == 0 else np.zeros_like(b_proj))
                        [None, :].astype(bf))
        in_maps.append(m)
    return in_maps


def kernel(x, W_qkv, b_qkv, W_proj, b_proj):
    x = np.asarray(x, dtype=np.float32)
    W_qkv = np.asarray(W_qkv, dtype=np.float32)
    b_qkv = np.asarray(b_qkv, dtype=np.float32)
    W_proj = np.asarray(W_proj, dtype=np.float32)
    b_proj = np.asarray(b_proj, dtype=np.float32)
    with_bias = bool(np.any(b_qkv) or np.any(b_proj))
    nc = _get_nc(with_bias)
    in_maps = _prep_in_maps(x, W_qkv, b_qkv, W_proj, b_proj, with_bias)
    res = run_bass_kernel_spmd(nc, in_maps, core_ids=list(range(N_CORES)))
    out = np.empty((B, N, C), dtype=np.float32)
    for b in range(B):
        out[b] = (res.results[2 * b]["out"].astype(np.float32)
                  + res.results[2 * b + 1]["out"].astype(np.float32))
    return out



# revision 28
# speedup vs baseline: 1.0952x; 1.0952x over previous
"""Trainium2 Bass kernel for multi-head self-attention (B=4, N=2048, C=1024, H=16).

Sharding: 8 cores = 4 batches x 2 head-groups (8 heads each). Per core:
  - V rows and Q^T/K^T packs from x[b] (PE), V interleaved with a per-head
    ones column so PV also produces softmax row sums
  - flash-style attention per (head, 1024-q chunk): S^T tiles -> exp ->
    PV with out layout [q, d+1] (full 128 output partitions)
  - normalize by 1/rowsum via per-partition tensor_scalar_mul, PE-transpose
    back to [d, q] for the output projection
Host preps per-core inputs and sums the two partial projections per batch.
"""

import numpy as np
import ml_dtypes

import concourse.bass as bass
import concourse.mybir as mybir
import concourse.tile as tile
from concourse import bacc
from concourse.ap import AP
from concourse.bass_utils import run_bass_kernel_spmd

BF16 = mybir.dt.bfloat16
F32 = mybir.dt.float32
FP8 = mybir.dt.float8e4
I16 = mybir.dt.int16
Exp = mybir.ActivationFunctionType.Exp
DR = mybir.MatmulPerfMode.DoubleRow
AluMult = mybir.AluOpType.mult
AluAdd = mybir.AluOpType.add
bf = ml_dtypes.bfloat16

B, N, C = 4, 2048, 1024
H, D = 16, 64
N_CORES = 8
HPC = H // 2     # heads per core (8)
PAIRS = HPC // 2  # head pairs per core (4)
CT = C // 128    # contraction tiles over C (8)
KT = N // 128    # key tiles (16)
RT = N // 128    # row tiles for V (16)
QT4 = N // 512   # 512-wide q chunks (4)
QC = 2           # 1024-wide q chunks
SCALE = 1.0 / float(np.sqrt(D))
# Schraudolph fast-exp in bf16-bit space: exp(s*SCALE) ~= bitcast_bf16(
# int16(round(s * FE_A + FE_B))). FE_B tuned for min RMS rel err (~1.8%).
FE_A = 128.0 * 1.4426950408889634 * SCALE
FE_B = 16248.5
# kt indices whose exp runs as DVE fast-exp in the 512-wide half
# segments (rest on ACT)
FAST_KT = frozenset((2, 5, 8, 11, 14))

import os
USE_FP8 = os.environ.get("K_FP8", "0") == "1"

_COMPILED = {}


def _build(with_bias: bool):
    nc = bacc.Bacc("TRN2", target_bir_lowering=False, debug=False,
                   num_devices=N_CORES)
    xt_d = nc.dram_tensor("xt", [C, N], BF16, kind="ExternalInput").ap()
    wqk_d = nc.dram_tensor("wqk", [C, 1024], BF16, kind="ExternalInput").ap()
    wv_d = nc.dram_tensor("wv", [C, 512], BF16, kind="ExternalInput").ap()
    wpr_d = nc.dram_tensor("wpr", [512, C], BF16, kind="ExternalInput").ap()
    id_d = nc.dram_tensor("ident", [128, 128], BF16, kind="ExternalInput").ap()
    if with_bias:
        bqk_d = nc.dram_tensor("bqk", [1, 1024], BF16, kind="ExternalInput").ap()
        bv_d = nc.dram_tensor("bv", [1, 512], BF16, kind="ExternalInput").ap()
        bpr_d = nc.dram_tensor("bpr", [1, C], BF16, kind="ExternalInput").ap()
    out_d = nc.dram_tensor("out", [N, C], BF16, kind="ExternalOutput").ap()

    with tile.TileContext(nc) as tc:
        with (
            tc.tile_pool(name="persist", bufs=1) as pp,
            tc.tile_pool(name="pt", bufs=24) as pt_pool,
            tc.tile_pool(name="st", bufs=3) as st_pool,
            tc.tile_pool(name="rc", bufs=6) as rc_pool,
            tc.tile_pool(name="q8s", bufs=4) as q8s_pool,
            tc.tile_pool(name="oc", bufs=4) as oc_pool,
            tc.tile_pool(name="ps_sc", bufs=2, space="PSUM") as ps_sc,
            tc.tile_pool(name="ps_pv", bufs=1, space="PSUM") as ps_pv,
            tc.tile_pool(name="ps_aux", bufs=2, space="PSUM") as ps_aux,
        ):
            # ---------------- input loads ----------------
            # alternate between the SP/HWDGE and Pool/SWDGE descriptor-gen
            # pipelines so bursts of DMAs generate in parallel
            dma_rr = [0]

            def dma(dst, srcap):
                # HWDGE gen is ~625ns/DMA vs SWDGE ~1040ns: weight 2:1
                eng = (nc.sync, nc.sync, nc.gpsimd)[dma_rr[0] % 3]
                dma_rr[0] += 1
                eng.dma_start(dst, srcap)

            xt_sb = [pp.tile([128, N], BF16, tag=f"xt{ct}", name=f"xt{ct}")
                     for ct in range(CT)]
            wv_sb = [pp.tile([128, 512], BF16, tag=f"wv{ct}", name=f"wv{ct}")
                     for ct in range(CT)]
            wqk_sb = [pp.tile([128, 1024], BF16, tag=f"wqk{ct}", name=f"wqk{ct}")
                      for ct in range(CT)]
            for ct in range(CT):
                dma(wv_sb[ct][:], wv_d[ct * 128:(ct + 1) * 128, :])
                dma(xt_sb[ct][:, 0:512], xt_d[ct * 128:(ct + 1) * 128, 0:512])
            for ct in range(CT):
                dma(wqk_sb[ct][:], wqk_d[ct * 128:(ct + 1) * 128, :])
            for q4 in range(1, QT4):
                qsl = slice(q4 * 512, (q4 + 1) * 512)
                for ct in range(CT):
                    dma(xt_sb[ct][:, qsl], xt_d[ct * 128:(ct + 1) * 128, qsl])
            wpr_sb = []
            for cp in range(PAIRS):
                t = pp.tile([128, C], BF16, tag=f"wpr{cp}", name=f"wpr{cp}")
                dma(t[:], wpr_d[cp * 128:(cp + 1) * 128, :])
                wpr_sb.append(t)
            ident = pp.tile([128, 128], BF16, tag="ident")
            dma(ident[:], id_d[:])

            def xt_s(ct, sl):
                return xt_sb[ct][:, sl]

            def wqk_s(ct, sl):
                return wqk_sb[ct][:, sl]

            def wv_s(ct):
                return wv_sb[ct][:]

            def wpr_s(cp, sl):
                return wpr_sb[cp][:, sl]
            if with_bias:
                ones = pp.tile([1, N], BF16, tag="ones")
                nc.vector.memset(ones[:], 1.0)
                bqk_sb = pp.tile([1, 1024], BF16, tag="bqk")
                dma(bqk_sb[:], bqk_d[:])
                bv_sb = pp.tile([1, 512], BF16, tag="bv")
                dma(bv_sb[:], bv_d[:])
                bpr_sb = pp.tile([1, C], BF16, tag="bpr")
                dma(bpr_sb[:], bpr_d[:])

            # ---------------- persistent working tiles ----------------
            if USE_FP8:
                # fp8 Q/K packs: tile p holds the pair's 2 heads at partition
                # bases 0/64 (base 96 is not addressable by matmul operands);
                # per head layout [32, 2(d-half), N]
                q8_sb = [pp.tile([128, 2 * N], FP8, tag=f"q8{p}", name=f"q8{p}")
                         for p in range(PAIRS)]
                k8_sb = [pp.tile([128, 2 * N], FP8, tag=f"k8{p}", name=f"k8{p}")
                         for p in range(PAIRS)]
            else:
                qt_sb = [pp.tile([128, N], BF16, tag=f"qt{p}", name=f"qt{p}")
                         for p in range(PAIRS)]
                kt_sb = [pp.tile([128, N], BF16, tag=f"kt{p}", name=f"kt{p}")
                         for p in range(PAIRS)]
            va_sb = [pp.tile([128, HPC * 65], BF16, tag=f"va{rt}", name=f"va{rt}")
                     for rt in range(RT)]
            plhsT = [pp.tile([128, N], BF16, tag=f"pl{p}", name=f"pl{p}")
                     for p in range(PAIRS)]

            # ---- V row tile: out[r, h*65+d] = sum_c xT[c, r] * Wv[c, h*64+d]
            v_acc = {}

            def v_tile_half(rt, half):
                va3 = va_sb[rt][:].rearrange("p (h d) -> p h d", d=65)
                rsl = slice(rt * 128, (rt + 1) * 128)
                if half == 0:
                    nc.vector.memset(va3[:, :, 64:65], 1.0)
                    v_acc[rt] = ps_aux.tile([128, 512], F32, tag="aux",
                                            name="acc_v")
                acc = v_acc[rt]
                for ct in range(4 * half, 4 * half + 4):
                    nc.tensor.matmul(acc[:], xt_s(ct, rsl), wv_s(ct),
                                     start=(ct == 0),
                                     stop=(not with_bias and ct == CT - 1))
                if half == 0:
                    return
                if with_bias:
                    nc.tensor.matmul(acc[:], ones[0:1, 0:128], bv_sb[:],
                                     start=False, stop=True)
                src3 = acc[:].rearrange("p (h d) -> p h d", d=64)
                nc.vector.tensor_copy(va3[:, :, 0:64], src3)
                del v_acc[rt]

            def v_tile(rt):
                v_tile_half(rt, 0)
                v_tile_half(rt, 1)

            # ---- Q^T/K^T pack chunk: out[o, q] = sum_c W[c, o] * xT[c, q]
            # split into two PE half-units so fillers stay fine-grained
            qk_acc = {}

            def qk_chunk_half(p, i, half):
                qk, q4 = i // QT4, i % QT4
                osl = slice(qk * 512 + p * 128, qk * 512 + (p + 1) * 128)
                qsl = slice(q4 * 512, (q4 + 1) * 512)
                if half == 0:
                    qk_acc[(p, i)] = ps_aux.tile([128, 512], F32, tag="aux",
                                                 name="acc_qk")
                acc = qk_acc[(p, i)]
                for ct in range(4 * half, 4 * half + 4):
                    nc.tensor.matmul(acc[:], wqk_s(ct, osl),
                                     xt_s(ct, qsl), start=(ct == 0),
                                     stop=(not with_bias and ct == CT - 1))
                if half == 0:
                    return
                if with_bias:
                    nc.tensor.matmul(acc[:], bqk_sb[0:1, osl], ones[0:1, qsl],
                                     start=False, stop=True)
                if USE_FP8:
                    stage = q8s_pool.tile([128, 512], FP8, tag="q8s",
                                          name="stage8")
                    nc.vector.tensor_copy(stage[:], acc[:])
                    dst = (q8_sb, k8_sb)[qk]
                    for hl in range(2):
                        for ih in range(2):
                            dma(dst[p][64 * hl:64 * hl + 32,
                                       ih * N + q4 * 512:ih * N + (q4 + 1) * 512],
                                stage[64 * hl + 32 * ih:64 * hl + 32 * ih + 32, :])
                else:
                    dst = (qt_sb, kt_sb)[qk]
                    nc.scalar.copy(dst[p][:, qsl], acc[:])
                del qk_acc[(p, i)]

            def qk_chunk(p, i):
                qk_chunk_half(p, i, 0)
                qk_chunk_half(p, i, 1)

            # two chunks with ct-interleaved matmuls: both chains advance as
            # each wqk tile lands (avoids head-of-line blocking on the
            # streaming weight DMAs)
            def qk_chunk2(p, ia, ib):
                accs = {}
                for i in (ia, ib):
                    accs[i] = ps_aux.tile([128, 512], F32, tag="aux",
                                          name="acc_qk")
                for ct in range(CT):
                    for i in (ia, ib):
                        qk, q4 = i // QT4, i % QT4
                        osl = slice(qk * 512 + p * 128,
                                    qk * 512 + (p + 1) * 128)
                        qsl = slice(q4 * 512, (q4 + 1) * 512)
                        nc.tensor.matmul(accs[i][:], wqk_s(ct, osl),
                                         xt_s(ct, qsl), start=(ct == 0),
                                         stop=(not with_bias and ct == CT - 1))
                for i in (ia, ib):
                    qk, q4 = i // QT4, i % QT4
                    osl = slice(qk * 512 + p * 128, qk * 512 + (p + 1) * 128)
                    qsl = slice(q4 * 512, (q4 + 1) * 512)
                    if with_bias:
                        nc.tensor.matmul(accs[i][:], bqk_sb[0:1, osl],
                                         ones[0:1, qsl], start=False, stop=True)
                    dst = (qt_sb, kt_sb)[qk]
                    nc.scalar.copy(dst[p][:, qsl], accs[i][:])

            # ---- partial output projection for one 128-row q tile (half) ----
            def proj_half(qt_i, nch):
                qsl = slice(qt_i * 128, (qt_i + 1) * 128)
                nsl = slice(nch * 512, (nch + 1) * 512)
                if nch == 0:
                    proj_oc[qt_i] = oc_pool.tile([128, 1024], BF16, tag="oc",
                                                 name="oc")
                oc = proj_oc[qt_i]
                acc = ps_aux.tile([128, 512], F32, tag="aux", name="acc_pr")
                for cp in range(PAIRS):
                    nc.tensor.matmul(acc[:], plhsT[cp][:, qsl],
                                     wpr_s(cp, nsl), start=(cp == 0),
                                     stop=(not with_bias and cp == PAIRS - 1))
                if with_bias:
                    nc.tensor.matmul(acc[:], ones[0:1, 0:128], bpr_sb[0:1, nsl],
                                     start=False, stop=True)
                if qt_i >= 14:
                    # tail-critical: put one half-copy on the idle ACT engine
                    # and fan the store over both DGE paths
                    (nc.vector.tensor_copy if nch == 0
                     else nc.scalar.copy)(oc[:, nsl], acc[:])
                    if nch == 1:
                        nc.sync.dma_start(out_d[qsl, 0:512], oc[:, 0:512])
                        nc.gpsimd.dma_start(out_d[qsl, 512:1024],
                                            oc[:, 512:1024])
                else:
                    # alternate copy engine to keep ACT/DVE balanced
                    (nc.vector.tensor_copy if (qt_i * 2 + nch) % 2 == 0
                     else nc.scalar.copy)(oc[:, nsl], acc[:])
                    if nch == 1:
                        dma(out_d[qsl, :], oc[:])

            proj_oc = {}

            # ---- attention segment: one head, one 1024-wide q chunk ----
            # pv accum regions: q-tile t<7 at col 65*t (bank 0), t=7 at col
            # 512 (bank 1). start=True on the first matmul into each bank
            # marks the whole bank pending-zero, so every region's first
            # write lands fresh; stop=True only on the bank's last matmul.
            pending_pe = []
            # decoupled PV stream: (exp_idx, closure). The softmax stream
            # (scores+exp) runs ahead; PV/drain/transpose work is pumped into
            # PE's spare capacity, bounded by the pt pool depth.
            pv_queue = []
            exp_idx = [0]
            PT_LAG = 14
            PT_BUFS = 16

            def pump_pv(n):
                for _ in range(n):
                    if pv_queue:
                        pv_queue.pop(0)[1]()

            # idle-slot pump: drain PV work but keep >=2 entries queued so
            # PV emission lags exp by ~2 kts and never eats exp latency
            def pump_keep(n, keep=2):
                for _ in range(n):
                    if len(pv_queue) > keep:
                        pv_queue.pop(0)[1]()

            def segment(h, qc, fillers, budget=1, inline_tr=False):
                p, hl = h // 2, h % 2
                g, r = h // 4, h % 4
                pv = ps_pv.tile([128, 1024], F32, tag="pv", name="pv")
                if hl == 0:
                    st_seg[p] = st_pool.tile([128, 1024], BF16, tag="st",
                                             name="st")
                st = st_seg[p]

                def pv_region(t):
                    c0 = 65 * t if t < 7 else 512
                    return pv[:, c0:c0 + 65]

                def scores(kt):
                    # two separate PSUM tiles (one per 512-wide matmul):
                    # PSUM tile deps are tile-granular, so separate tiles
                    # keep the ACT and DVE exp readers fully decoupled
                    ksl = slice(kt * 128, (kt + 1) * 128)
                    out = []
                    for qh in range(2):
                        q0 = qc * 1024 + qh * 512
                        sc = ps_sc.tile([128, 512], F32,
                                        tag=("sca", "scb")[qh], name="sc")
                        nc.tensor.matmul(
                            sc[:],
                            kt_sb[p][64 * hl:64 * hl + 64, ksl],
                            qt_sb[p][64 * hl:64 * hl + 64, q0:q0 + 512],
                            start=True, stop=True)
                        out.append(sc)
                    return out

                def pv_group(kt, pt, ptb):
                    for t in range(8):
                        lhsT = (pt[:, 128 * t:128 * (t + 1)] if t < 4
                                else ptb[:, 128 * (t - 4):128 * (t - 3)])
                        nc.tensor.matmul(
                            pv_region(t), lhsT,
                            va_sb[kt][:, h * 65:(h + 1) * 65],
                            start=(kt == 0 and t in (0, 7)),
                            stop=(kt == KT - 1),
                            skip_group_check=True)

                scs = [scores(0), scores(1)]
                for kt in range(KT):
                    # pt-slot WAR: the PV group reading the tile this exp's
                    # slot will recycle must be emitted before the exp
                    while pv_queue and (pv_queue[0][0] is None or
                                        pv_queue[0][0] <= exp_idx[0] - PT_LAG):
                        pump_pv(1)
                    # exp split by half across both engines: ACT does an
                    # exact Exp on sc_a, DVE a Schraudolph fast-exp on sc_b
                    # (int16 round of s*FE_A+FE_B, bitcast to bf16). Each
                    # half's sc tile has exactly one reader, so the WAR for
                    # the kt+2 scores matmul waits only its own 512-wide exp
                    pt = pt_pool.tile([128, 512], BF16, tag="pt", name="pt",
                                      bufs=PT_BUFS)
                    ptb = pt_pool.tile([128, 512], BF16, tag="ptb",
                                       name="ptb", bufs=PT_BUFS)
                    sca, scb = scs[kt % 2]
                    nc.scalar.activation(pt[:], sca[:], Exp, scale=SCALE)
                    nc.vector.tensor_scalar(
                        ptb[:].bitcast(I16), scb[:],
                        FE_A, FE_B, AluMult, AluAdd)
                    # fillers/PV first: they are ready PE work that rides
                    # out the exp latency before the in-order PE queue hits
                    # the WAR-gated lookahead scores
                    if kt < 6:
                        if fillers:
                            for _ in range(budget):
                                if fillers:
                                    fillers.pop(0)()
                        elif pending_pe:
                            pending_pe.pop(0)()
                        else:
                            pump_keep(2)
                    else:
                        if pending_pe:
                            pending_pe.pop(0)()
                        elif fillers:
                            for _ in range(budget):
                                if fillers:
                                    fillers.pop(0)()
                        else:
                            pump_keep(2)
                    if kt + 2 < KT:
                        scs[kt % 2] = scores(kt + 2)
                    pv_queue.append((exp_idx[0], lambda kt=kt, pt=pt, ptb=ptb:
                                     pv_group(kt, pt, ptb)))
                    exp_idx[0] += 1

                # normalize: recip of row sums, per-partition scalar multiply
                def drain(pv=pv, st=st, hl=hl):
                    recip = rc_pool.tile([128, 8], F32, tag="rc", name="recip")
                    nc.vector.reciprocal(recip[:, 0:7], pv[:, 64:455:65])
                    st3 = st[:].rearrange("p (t d) -> p t d", d=128)[
                        :, 0:7, 64 * hl:64 * hl + 64]
                    pv3 = pv[:, 0:455].rearrange("p (t d) -> p t d", d=65)[
                        :, :, 0:64]
                    rc3 = recip[:, 0:7].rearrange("p (t d) -> p t d", d=1)
                    nc.vector.tensor_mul(st3, pv3,
                                         rc3.broadcast_to((128, 7, 64)))
                    nc.vector.reciprocal(recip[:, 7:8], pv[:, 576:577])
                    nc.vector.tensor_scalar_mul(
                        st[:, 128 * 7 + 64 * hl:128 * 7 + 64 * hl + 64],
                        pv[:, 512:576], recip[:, 7:8])

                pv_queue.append((None, drain))
                if hl == 1:
                    # transpose [q, d-pair] -> [d-pair, q] via PE, then one
                    # copy into the proj lhsT pack; deferred into the next
                    # segment's filler slots so the PE stream never stalls
                    # on the DVE normalize chain
                    tr_box = {}

                    def tr_half(lo, st=st, box=tr_box):
                        if lo == 0:
                            box["aux"] = ps_aux.tile([128, 512], F32,
                                                     tag="aux", name="tr")
                        tr = box["aux"].bitcast(BF16)
                        for t in range(lo, lo + 4):
                            nc.tensor.transpose(tr[:, 128 * t:128 * (t + 1)],
                                                st[:, 128 * t:128 * (t + 1)],
                                                ident[:])

                    def tr_copy(p=p, qc=qc, box=tr_box):
                        nc.vector.tensor_copy(
                            plhsT[p][:, qc * 1024:(qc + 1) * 1024],
                            box["aux"].bitcast(BF16))

                    pv_queue.append((None, lambda: tr_half(0)))
                    pv_queue.append((None, lambda: tr_half(4)))
                    pv_queue.append((None, tr_copy))
                if inline_tr:
                    pump_pv(len(pv_queue))

            st_seg = {}

            # ---- 512-wide half segment: used to split the very last
            # (head, q-chunk) so the final projection tiles overlap the
            # second half's softmax stream
            def half_segment(h, qc, qh, fillers):
                p, hl = h // 2, h % 2
                pv = ps_pv.tile([128, 1024], F32, tag="pv", name="pv")
                if hl == 0 and qh == 0:
                    st_seg[p] = st_pool.tile([128, 1024], BF16, tag="st",
                                             name="st")
                st = st_seg[p]
                q0 = qc * 1024 + qh * 512

                def scores(kt):
                    sc = ps_sc.tile([128, 512], F32,
                                    tag=("sca", "scb")[kt % 2], name="sc")
                    ksl = slice(kt * 128, (kt + 1) * 128)
                    nc.tensor.matmul(
                        sc[:],
                        kt_sb[p][64 * hl:64 * hl + 64, ksl],
                        qt_sb[p][64 * hl:64 * hl + 64, q0:q0 + 512],
                        start=True, stop=True)
                    return sc

                scs = [scores(0), scores(1)]
                for kt in range(KT):
                    pt = pt_pool.tile([128, 512], BF16, tag="pth", name="pt",
                                      bufs=6)
                    if kt in FAST_KT:
                        nc.vector.tensor_scalar(pt[:].bitcast(I16),
                                                scs[kt % 2][:],
                                                FE_A, FE_B, AluMult, AluAdd)
                    else:
                        nc.scalar.activation(pt[:],
                                             scs[kt % 2][:],
                                             Exp, scale=SCALE)
                    if fillers:
                        fillers.pop(0)()
                    if kt + 2 < KT:
                        scs[kt % 2] = scores(kt + 2)
                    for t in range(4):
                        nc.tensor.matmul(
                            pv[:, 65 * t:65 * t + 65],
                            pt[:, 128 * t:128 * (t + 1)],
                            va_sb[kt][:, h * 65:(h + 1) * 65],
                            start=(kt == 0 and t == 0),
                            stop=(kt == KT - 1),
                            skip_group_check=True)

                recip = rc_pool.tile([128, 8], F32, tag="rc", name="recip")
                nc.vector.reciprocal(recip[:, 0:4], pv[:, 64:260:65])
                st3 = st[:].rearrange("p (t d) -> p t d", d=128)[
                    :, 4 * qh:4 * qh + 4, 64 * hl:64 * hl + 64]
                pv3 = pv[:, 0:260].rearrange("p (t d) -> p t d", d=65)[
                    :, :, 0:64]
                rc3 = recip[:, 0:4].rearrange("p (t d) -> p t d", d=1)
                nc.vector.tensor_mul(st3, pv3, rc3.broadcast_to((128, 4, 64)))
                if hl == 1:
                    aux = ps_aux.tile([128, 512], F32, tag="aux", name="tr")
                    tr = aux.bitcast(BF16)
                    for t in range(4):
                        nc.tensor.transpose(
                            tr[:, 128 * t:128 * (t + 1)],
                            st[:, 128 * (4 * qh + t):128 * (4 * qh + t + 1)],
                            ident[:])
                    nc.vector.tensor_copy(
                        plhsT[p][:, q0:q0 + 512], tr[:, 0:512])

            # ---------------- emission schedule ----------------
            # head: only what the first exps need (V rows 0-1, pair 0's
            # low-q Q packs and first K chunk); everything else fills
            # segment slots just-in-time
            v_tile(0)
            v_tile(1)
            v_tile(2)
            if USE_FP8:
                for i in (0, 1, 4):
                    qk_chunk(0, i)
            else:
                qk_chunk2(0, 0, 1)
                qk_chunk(0, 4)

            def pack_units(p, chunks):
                return [lambda p=p, i=i, half=half: qk_chunk_half(p, i, half)
                        for i in chunks for half in range(2)]

            def v_units(rts):
                return [lambda rt=rt, half=half: v_tile_half(rt, half)
                        for rt in rts for half in range(2)]

            # per-segment fillers, sized ~0.5-1 exp each, due before use:
            # seg 0 runs at 2 pops/kt to finish the V rows just-in-time;
            # pair p's K + low-q Q packs before seg 2p; high-q Q (needed by
            # qc1 only) in segs 5-8; projection of qc0 rows spread over qc1
            seg_fill = {i: [] for i in range(2 * HPC)}
            seg_fill[0] = (v_units([3]) + pack_units(0, [5])
                           + v_units([4, 5, 6]) + pack_units(0, [6])
                           + v_units([7, 8, 9]) + pack_units(0, [7])
                           + v_units([10, 11, 12, 13, 14, 15]))
            seg_fill[1] = pack_units(1, [0, 1, 4])
            seg_fill[2] = pack_units(1, [5, 6, 7]) + pack_units(2, [0])
            seg_fill[3] = pack_units(2, [1, 4]) + pack_units(3, [0])
            seg_fill[4] = pack_units(2, [5, 6, 7]) + pack_units(3, [1])
            seg_fill[5] = pack_units(3, [4]) + pack_units(0, [2, 3])
            seg_fill[6] = pack_units(3, [5, 6, 7])
            seg_fill[7] = pack_units(1, [2, 3])
            seg_fill[8] = pack_units(2, [2, 3])
            seg_fill[9] = pack_units(3, [2, 3])
            for s in range(HPC):
                seg_fill[8 + s] += [
                    lambda qt_i=s, nch=j: proj_half(qt_i, nch)
                    for j in range(2)]

            carry = []
            for qc in range(QC):
                for h in range(HPC):
                    if qc == 1 and h == HPC - 1:
                        break
                    fl = carry + seg_fill[qc * HPC + h]
                    segment(h, qc, fl,
                            budget=2 if (qc == 0 and h == 0) else 1,
                            inline_tr=(qc == 0 and h == HPC - 1))
                    carry = fl
            pump_pv(len(pv_queue))
            half_segment(HPC - 1, 1, 0, carry + seg_fill[15])
            tail_fill = [lambda qt_i=8 + (j // 2), nch=j % 2:
                         proj_half(qt_i, nch) for j in range(8)]
            half_segment(HPC - 1, 1, 1, tail_fill)
            for w in pending_pe:
                w()
            pending_pe.clear()
            for qt_i in range(12, 16):
                proj_half(qt_i, 0)
                proj_half(qt_i, 1)

    nc.compile()
    return nc


def _get_nc(with_bias=False):
    if with_bias not in _COMPILED:
        _COMPILED[with_bias] = _build(with_bias)
    return _COMPILED[with_bias]


def _prep_in_maps(x, W_qkv, b_qkv, W_proj, b_proj, with_bias):
    ident = np.eye(128).astype(bf)
    in_maps = []
    for c in range(N_CORES):
        b = c // 2
        g = c % 2
        hs = slice(g * 512, (g + 1) * 512)
        xt = np.ascontiguousarray(x[b].T).astype(bf)
        wq = W_qkv[:, 0:C][:, hs]
        wk = W_qkv[:, C:2 * C][:, hs]
        wv = W_qkv[:, 2 * C:3 * C][:, hs]
        wqk = np.ascontiguousarray(np.concatenate([wq, wk], axis=1)).astype(bf)
        wpr = np.ascontiguousarray(W_proj[hs, :]).astype(bf)
        m = {
            "xt": xt, "wqk": wqk, "wv": np.ascontiguousarray(wv).astype(bf),
            "wpr": wpr, "ident": ident,
        }
        if with_bias:
            bq = b_qkv[0:C][hs]
            bk = b_qkv[C:2 * C][hs]
            bvv = b_qkv[2 * C:3 * C][hs]
            m["bqk"] = np.concatenate([bq, bk])[None, :].astype(bf)
            m["bv"] = np.ascontiguousarray(bvv[None, :]).astype(bf)
            m["bpr"] = ((b_proj if g == 0 else np.zeros_like(b_proj))
                        [None, :].astype(bf))
        in_maps.append(m)
    return in_maps


def kernel(x, W_qkv, b_qkv, W_proj, b_proj):
    x = np.asarray(x, dtype=np.float32)
    W_qkv = np.asarray(W_qkv, dtype=np.float32)
    b_qkv = np.asarray(b_qkv, dtype=np.float32)
    W_proj = np.asarray(W_proj, dtype=np.float32)
    b_proj = np.asarray(b_proj, dtype=np.float32)
    with_bias = bool(np.any(b_qkv) or np.any(b_proj))
    nc = _get_nc(with_bias)
    in_maps = _prep_in_maps(x, W_qkv, b_qkv, W_proj, b_proj, with_bias)
    res = run_bass_kernel_spmd(nc, in_maps, core_ids=list(range(N_CORES)))
    out = np.empty((B, N, C), dtype=np.float32)
    for b in range(B):
        out[b] = (res.results[2 * b]["out"].astype(np.float32)
                  + res.results[2 * b + 1]["out"].astype(np.float32))
    return out



# revision 45
# speedup vs baseline: 1.1019x; 1.0061x over previous
"""Trainium2 Bass kernel for multi-head self-attention (B=4, N=2048, C=1024, H=16).

Sharding: 8 cores = 4 batches x 2 head-groups (8 heads each). Per core:
  - V rows and Q^T/K^T packs from x[b] (PE), V interleaved with a per-head
    ones column so PV also produces softmax row sums
  - flash-style attention per (head, 1024-q chunk): S^T tiles -> exp ->
    PV with out layout [q, d+1] (full 128 output partitions)
  - normalize by 1/rowsum via per-partition tensor_scalar_mul, PE-transpose
    back to [d, q] for the output projection
Host preps per-core inputs and sums the two partial projections per batch.
"""

import numpy as np
import ml_dtypes

import concourse.bass as bass
import concourse.mybir as mybir
import concourse.tile as tile
from concourse import bacc
from concourse.ap import AP
from concourse.bass_utils import run_bass_kernel_spmd

BF16 = mybir.dt.bfloat16
F32 = mybir.dt.float32
FP8 = mybir.dt.float8e4
I16 = mybir.dt.int16
Exp = mybir.ActivationFunctionType.Exp
DR = mybir.MatmulPerfMode.DoubleRow
AluMult = mybir.AluOpType.mult
AluAdd = mybir.AluOpType.add
bf = ml_dtypes.bfloat16

B, N, C = 4, 2048, 1024
H, D = 16, 64
N_CORES = 8
HPC = H // 2     # heads per core (8)
PAIRS = HPC // 2  # head pairs per core (4)
CT = C // 128    # contraction tiles over C (8)
KT = N // 128    # key tiles (16)
RT = N // 128    # row tiles for V (16)
QT4 = N // 512   # 512-wide q chunks (4)
QC = 2           # 1024-wide q chunks
SCALE = 1.0 / float(np.sqrt(D))
# Schraudolph fast-exp in bf16-bit space: exp(s*SCALE) ~= bitcast_bf16(
# int16(round(s * FE_A + FE_B))). FE_B tuned for min RMS rel err (~1.8%).
FE_A = 128.0 * 1.4426950408889634 * SCALE
FE_B = 16248.5
# kt indices whose exp runs as DVE fast-exp in the 512-wide half
# segments (rest on ACT)
FAST_KT = frozenset((2, 5, 8, 11, 14))

import os
USE_FP8 = os.environ.get("K_FP8", "0") == "1"

_COMPILED = {}


def _build(with_bias: bool):
    nc = bacc.Bacc("TRN2", target_bir_lowering=False, debug=False,
                   num_devices=N_CORES)
    xt_d = nc.dram_tensor("xt", [C, N], BF16, kind="ExternalInput").ap()
    wqk_d = nc.dram_tensor("wqk", [C, 1024], BF16, kind="ExternalInput").ap()
    wv_d = nc.dram_tensor("wv", [C, 512], BF16, kind="ExternalInput").ap()
    wpr_d = nc.dram_tensor("wpr", [512, C], BF16, kind="ExternalInput").ap()
    id_d = nc.dram_tensor("ident", [128, 128], BF16, kind="ExternalInput").ap()
    if with_bias:
        bqk_d = nc.dram_tensor("bqk", [1, 1024], BF16, kind="ExternalInput").ap()
        bv_d = nc.dram_tensor("bv", [1, 512], BF16, kind="ExternalInput").ap()
        bpr_d = nc.dram_tensor("bpr", [1, C], BF16, kind="ExternalInput").ap()
    out_d = nc.dram_tensor("out", [N, C], BF16, kind="ExternalOutput").ap()

    with tile.TileContext(nc) as tc:
        with (
            tc.tile_pool(name="persist", bufs=1) as pp,
            tc.tile_pool(name="pt", bufs=24) as pt_pool,
            tc.tile_pool(name="st", bufs=3) as st_pool,
            tc.tile_pool(name="rc", bufs=6) as rc_pool,
            tc.tile_pool(name="q8s", bufs=4) as q8s_pool,
            tc.tile_pool(name="oc", bufs=4) as oc_pool,
            tc.tile_pool(name="ps_sc", bufs=2, space="PSUM") as ps_sc,
            tc.tile_pool(name="ps_pv", bufs=1, space="PSUM") as ps_pv,
            tc.tile_pool(name="ps_aux", bufs=2, space="PSUM") as ps_aux,
        ):
            # ---------------- input loads ----------------
            # alternate between the SP/HWDGE and Pool/SWDGE descriptor-gen
            # pipelines so bursts of DMAs generate in parallel
            dma_rr = [0]
            # first four loads (wv0,xt0,wv1,xt1) feed the very first PE
            # matmuls: split them across both descriptor-gen queues so the
            # first pair lands ~1us earlier than a single serial queue
            _head_eng = [nc.gpsimd, nc.sync, nc.sync, nc.gpsimd]

            def dma(dst, srcap):
                # HWDGE gen is ~625ns/DMA vs SWDGE ~1040ns: weight 2:1
                if dma_rr[0] < len(_head_eng):
                    eng = _head_eng[dma_rr[0]]
                else:
                    eng = (nc.sync, nc.sync, nc.gpsimd)[dma_rr[0] % 3]
                dma_rr[0] += 1
                eng.dma_start(dst, srcap)

            xt_sb = [pp.tile([128, N], BF16, tag=f"xt{ct}", name=f"xt{ct}")
                     for ct in range(CT)]
            wv_sb = [pp.tile([128, 512], BF16, tag=f"wv{ct}", name=f"wv{ct}")
                     for ct in range(CT)]
            wqk_sb = [pp.tile([128, 1024], BF16, tag=f"wqk{ct}", name=f"wqk{ct}")
                      for ct in range(CT)]
            for ct in range(CT):
                dma(wv_sb[ct][:], wv_d[ct * 128:(ct + 1) * 128, :])
                dma(xt_sb[ct][:, 0:512], xt_d[ct * 128:(ct + 1) * 128, 0:512])
            # wqk interleaved with the q4=1 x columns: the head's Q/K pack
            # chunks consume exactly (wqk[ct], xt[ct][512:1024]) per ct step
            for ct in range(CT):
                dma(wqk_sb[ct][:], wqk_d[ct * 128:(ct + 1) * 128, :])
                dma(xt_sb[ct][:, 512:1024],
                    xt_d[ct * 128:(ct + 1) * 128, 512:1024])
            for q4 in range(2, QT4):
                qsl = slice(q4 * 512, (q4 + 1) * 512)
                for ct in range(CT):
                    dma(xt_sb[ct][:, qsl], xt_d[ct * 128:(ct + 1) * 128, qsl])
            wpr_sb = []
            for cp in range(PAIRS):
                t = pp.tile([128, C], BF16, tag=f"wpr{cp}", name=f"wpr{cp}")
                dma(t[:], wpr_d[cp * 128:(cp + 1) * 128, :])
                wpr_sb.append(t)
            ident = pp.tile([128, 128], BF16, tag="ident")
            dma(ident[:], id_d[:])

            def xt_s(ct, sl):
                return xt_sb[ct][:, sl]

            def wqk_s(ct, sl):
                return wqk_sb[ct][:, sl]

            def wv_s(ct):
                return wv_sb[ct][:]

            def wpr_s(cp, sl):
                return wpr_sb[cp][:, sl]
            if with_bias:
                ones = pp.tile([1, N], BF16, tag="ones")
                nc.vector.memset(ones[:], 1.0)
                bqk_sb = pp.tile([1, 1024], BF16, tag="bqk")
                dma(bqk_sb[:], bqk_d[:])
                bv_sb = pp.tile([1, 512], BF16, tag="bv")
                dma(bv_sb[:], bv_d[:])
                bpr_sb = pp.tile([1, C], BF16, tag="bpr")
                dma(bpr_sb[:], bpr_d[:])

            # ---------------- persistent working tiles ----------------
            if USE_FP8:
                # fp8 Q/K packs: tile p holds the pair's 2 heads at partition
                # bases 0/64 (base 96 is not addressable by matmul operands);
                # per head layout [32, 2(d-half), N]
                q8_sb = [pp.tile([128, 2 * N], FP8, tag=f"q8{p}", name=f"q8{p}")
                         for p in range(PAIRS)]
                k8_sb = [pp.tile([128, 2 * N], FP8, tag=f"k8{p}", name=f"k8{p}")
                         for p in range(PAIRS)]
            else:
                qt_sb = [pp.tile([128, N], BF16, tag=f"qt{p}", name=f"qt{p}")
                         for p in range(PAIRS)]
                kt_sb = [pp.tile([128, N], BF16, tag=f"kt{p}", name=f"kt{p}")
                         for p in range(PAIRS)]
            va_sb = [pp.tile([128, HPC * 65], BF16, tag=f"va{rt}", name=f"va{rt}")
                     for rt in range(RT)]
            plhsT = [pp.tile([128, N], BF16, tag=f"pl{p}", name=f"pl{p}")
                     for p in range(PAIRS)]

            # ---- V row tile: out[r, h*65+d] = sum_c xT[c, r] * Wv[c, h*64+d]
            v_acc = {}

            def v_tile_half(rt, half):
                va3 = va_sb[rt][:].rearrange("p (h d) -> p h d", d=65)
                rsl = slice(rt * 128, (rt + 1) * 128)
                if half == 0:
                    nc.vector.memset(va3[:, :, 64:65], 1.0)
                    v_acc[rt] = ps_aux.tile([128, 512], F32, tag="aux",
                                            name="acc_v")
                acc = v_acc[rt]
                for ct in range(4 * half, 4 * half + 4):
                    nc.tensor.matmul(acc[:], xt_s(ct, rsl), wv_s(ct),
                                     start=(ct == 0),
                                     stop=(not with_bias and ct == CT - 1))
                if half == 0:
                    return
                if with_bias:
                    nc.tensor.matmul(acc[:], ones[0:1, 0:128], bv_sb[:],
                                     start=False, stop=True)
                src3 = acc[:].rearrange("p (h d) -> p h d", d=64)
                nc.vector.tensor_copy(va3[:, :, 0:64], src3)
                del v_acc[rt]

            def v_tile(rt):
                v_tile_half(rt, 0)
                v_tile_half(rt, 1)

            # ---- Q^T/K^T pack chunk: out[o, q] = sum_c W[c, o] * xT[c, q]
            # split into two PE half-units so fillers stay fine-grained
            qk_acc = {}

            def qk_chunk_half(p, i, half):
                qk, q4 = i // QT4, i % QT4
                osl = slice(qk * 512 + p * 128, qk * 512 + (p + 1) * 128)
                qsl = slice(q4 * 512, (q4 + 1) * 512)
                if half == 0:
                    qk_acc[(p, i)] = ps_aux.tile([128, 512], F32, tag="aux",
                                                 name="acc_qk")
                acc = qk_acc[(p, i)]
                for ct in range(4 * half, 4 * half + 4):
                    nc.tensor.matmul(acc[:], wqk_s(ct, osl),
                                     xt_s(ct, qsl), start=(ct == 0),
                                     stop=(not with_bias and ct == CT - 1))
                if half == 0:
                    return
                if with_bias:
                    nc.tensor.matmul(acc[:], bqk_sb[0:1, osl], ones[0:1, qsl],
                                     start=False, stop=True)
                if USE_FP8:
                    stage = q8s_pool.tile([128, 512], FP8, tag="q8s",
                                          name="stage8")
                    nc.vector.tensor_copy(stage[:], acc[:])
                    dst = (q8_sb, k8_sb)[qk]
                    for hl in range(2):
                        for ih in range(2):
                            dma(dst[p][64 * hl:64 * hl + 32,
                                       ih * N + q4 * 512:ih * N + (q4 + 1) * 512],
                                stage[64 * hl + 32 * ih:64 * hl + 32 * ih + 32, :])
                else:
                    dst = (qt_sb, kt_sb)[qk]
                    nc.scalar.copy(dst[p][:, qsl], acc[:])
                del qk_acc[(p, i)]

            def qk_chunk(p, i):
                qk_chunk_half(p, i, 0)
                qk_chunk_half(p, i, 1)

            # two chunks with ct-interleaved matmuls: both chains advance as
            # each wqk tile lands (avoids head-of-line blocking on the
            # streaming weight DMAs)
            def qk_chunk2(p, ia, ib):
                accs = {}
                for i in (ia, ib):
                    accs[i] = ps_aux.tile([128, 512], F32, tag="aux",
                                          name="acc_qk")
                for ct in range(CT):
                    for i in (ia, ib):
                        qk, q4 = i // QT4, i % QT4
                        osl = slice(qk * 512 + p * 128,
                                    qk * 512 + (p + 1) * 128)
                        qsl = slice(q4 * 512, (q4 + 1) * 512)
                        nc.tensor.matmul(accs[i][:], wqk_s(ct, osl),
                                         xt_s(ct, qsl), start=(ct == 0),
                                         stop=(not with_bias and ct == CT - 1))
                for i in (ia, ib):
                    qk, q4 = i // QT4, i % QT4
                    osl = slice(qk * 512 + p * 128, qk * 512 + (p + 1) * 128)
                    qsl = slice(q4 * 512, (q4 + 1) * 512)
                    if with_bias:
                        nc.tensor.matmul(accs[i][:], bqk_sb[0:1, osl],
                                         ones[0:1, qsl], start=False, stop=True)
                    dst = (qt_sb, kt_sb)[qk]
                    nc.scalar.copy(dst[p][:, qsl], accs[i][:])

            # ---- partial output projection for one 128-row q tile (half) ----
            def proj_half(qt_i, nch):
                qsl = slice(qt_i * 128, (qt_i + 1) * 128)
                nsl = slice(nch * 512, (nch + 1) * 512)
                if nch == 0:
                    proj_oc[qt_i] = oc_pool.tile([128, 1024], BF16, tag="oc",
                                                 name="oc")
                oc = proj_oc[qt_i]
                acc = ps_aux.tile([128, 512], F32, tag="aux", name="acc_pr")
                for cp in range(PAIRS):
                    nc.tensor.matmul(acc[:], plhsT[cp][:, qsl],
                                     wpr_s(cp, nsl), start=(cp == 0),
                                     stop=(not with_bias and cp == PAIRS - 1))
                if with_bias:
                    nc.tensor.matmul(acc[:], ones[0:1, 0:128], bpr_sb[0:1, nsl],
                                     start=False, stop=True)
                if qt_i >= 12:
                    # tail-critical: put one half-copy on the idle ACT engine
                    # and store with a single descriptor-gen pass (the HWDGE
                    # generator is one serial device; two 512-wide stores
                    # would serialize their gens on the critical tail)
                    (nc.vector.tensor_copy if nch == 0
                     else nc.scalar.copy)(oc[:, nsl], acc[:])
                    if nch == 1:
                        nc.sync.dma_start(out_d[qsl, :], oc[:])
                else:
                    # alternate copy engine to keep ACT/DVE balanced
                    (nc.vector.tensor_copy if (qt_i * 2 + nch) % 2 == 0
                     else nc.scalar.copy)(oc[:, nsl], acc[:])
                    if nch == 1:
                        dma(out_d[qsl, :], oc[:])

            proj_oc = {}

            # ---- attention segment: one head, one 1024-wide q chunk ----
            # pv accum regions: q-tile t<7 at col 65*t (bank 0), t=7 at col
            # 512 (bank 1). start=True on the first matmul into each bank
            # marks the whole bank pending-zero, so every region's first
            # write lands fresh; stop=True only on the bank's last matmul.
            pending_pe = []
            # decoupled PV stream: (exp_idx, closure). The softmax stream
            # (scores+exp) runs ahead; PV/drain/transpose work is pumped into
            # PE's spare capacity, bounded by the pt pool depth.
            pv_queue = []
            exp_idx = [0]
            PT_LAG = 14
            PT_BUFS = 16

            def pump_pv(n):
                for _ in range(n):
                    if pv_queue:
                        pv_queue.pop(0)[1]()

            # idle-slot pump: drain PV work but keep >=2 entries queued so
            # PV emission lags exp by ~2 kts and never eats exp latency
            def pump_keep(n, keep=2):
                for _ in range(n):
                    if len(pv_queue) > keep:
                        pv_queue.pop(0)[1]()

            def segment(h, qc, fillers, budget=1, inline_tr=False):
                p, hl = h // 2, h % 2
                g, r = h // 4, h % 4
                pv = ps_pv.tile([128, 1024], F32, tag="pv", name="pv")
                if hl == 0:
                    st_seg[p] = st_pool.tile([128, 1024], BF16, tag="st",
                                             name="st")
                st = st_seg[p]

                def pv_region(t):
                    c0 = 65 * t if t < 7 else 512
                    return pv[:, c0:c0 + 65]

                def scores(kt):
                    # two separate PSUM tiles (one per 512-wide matmul):
                    # PSUM tile deps are tile-granular, so separate tiles
                    # keep the ACT and DVE exp readers fully decoupled
                    ksl = slice(kt * 128, (kt + 1) * 128)
                    out = []
                    for qh in range(2):
                        q0 = qc * 1024 + qh * 512
                        sc = ps_sc.tile([128, 512], F32,
                                        tag=("sca", "scb")[qh], name="sc")
                        nc.tensor.matmul(
                            sc[:],
                            kt_sb[p][64 * hl:64 * hl + 64, ksl],
                            qt_sb[p][64 * hl:64 * hl + 64, q0:q0 + 512],
                            start=True, stop=True)
                        out.append(sc)
                    return out

                def pv_group(kt, pt, ptb):
                    for t in range(8):
                        lhsT = (pt[:, 128 * t:128 * (t + 1)] if t < 4
                                else ptb[:, 128 * (t - 4):128 * (t - 3)])
                        nc.tensor.matmul(
                            pv_region(t), lhsT,
                            va_sb[kt][:, h * 65:(h + 1) * 65],
                            start=(kt == 0 and t in (0, 7)),
                            stop=(kt == KT - 1),
                            skip_group_check=True)

                scs = [scores(0), scores(1)]
                for kt in range(KT):
                    # pt-slot WAR: the PV group reading the tile this exp's
                    # slot will recycle must be emitted before the exp
                    while pv_queue and (pv_queue[0][0] is None or
                                        pv_queue[0][0] <= exp_idx[0] - PT_LAG):
                        pump_pv(1)
                    # exp split by half across both engines: ACT does an
                    # exact Exp on sc_a, DVE a Schraudolph fast-exp on sc_b
                    # (int16 round of s*FE_A+FE_B, bitcast to bf16). Each
                    # half's sc tile has exactly one reader, so the WAR for
                    # the kt+2 scores matmul waits only its own 512-wide exp
                    pt = pt_pool.tile([128, 512], BF16, tag="pt", name="pt",
                                      bufs=PT_BUFS)
                    ptb = pt_pool.tile([128, 512], BF16, tag="ptb",
                                       name="ptb", bufs=PT_BUFS)
                    sca, scb = scs[kt % 2]
                    # fillers first: their ACT/DVE copies enqueue ahead of
                    # this kt's exps (filler kts have ample chain slack), so
                    # pack tiles land an exp earlier for the segments that
                    # consume them; their PE matmuls also ride out the exp
                    # latency before the WAR-gated lookahead scores
                    if kt < 6:
                        if fillers:
                            for _ in range(budget):
                                if fillers:
                                    fillers.pop(0)()
                        elif pending_pe:
                            pending_pe.pop(0)()
                        else:
                            pump_keep(2)
                    else:
                        if pending_pe:
                            pending_pe.pop(0)()
                        elif fillers:
                            for _ in range(budget):
                                if fillers:
                                    fillers.pop(0)()
                        else:
                            pump_keep(2)
                    nc.scalar.activation(pt[:], sca[:], Exp, scale=SCALE)
                    nc.vector.tensor_scalar(
                        ptb[:].bitcast(I16), scb[:],
                        FE_A, FE_B, AluMult, AluAdd)
                    if kt + 2 < KT:
                        scs[kt % 2] = scores(kt + 2)
                    pv_queue.append((exp_idx[0], lambda kt=kt, pt=pt, ptb=ptb:
                                     pv_group(kt, pt, ptb)))
                    exp_idx[0] += 1

                # normalize: recip of row sums, per-partition scalar multiply
                def drain(pv=pv, st=st, hl=hl):
                    recip = rc_pool.tile([128, 8], F32, tag="rc", name="recip")
                    nc.vector.reciprocal(recip[:, 0:7], pv[:, 64:455:65])
                    st3 = st[:].rearrange("p (t d) -> p t d", d=128)[
                        :, 0:7, 64 * hl:64 * hl + 64]
                    pv3 = pv[:, 0:455].rearrange("p (t d) -> p t d", d=65)[
                        :, :, 0:64]
                    rc3 = recip[:, 0:7].rearrange("p (t d) -> p t d", d=1)
                    nc.vector.tensor_mul(st3, pv3,
                                         rc3.broadcast_to((128, 7, 64)))
                    nc.vector.reciprocal(recip[:, 7:8], pv[:, 576:577])
                    nc.vector.tensor_scalar_mul(
                        st[:, 128 * 7 + 64 * hl:128 * 7 + 64 * hl + 64],
                        pv[:, 512:576], recip[:, 7:8])

                pv_queue.append((None, drain))
                if hl == 1:
                    # transpose [q, d-pair] -> [d-pair, q] via PE, then one
                    # copy into the proj lhsT pack; deferred into the next
                    # segment's filler slots so the PE stream never stalls
                    # on the DVE normalize chain
                    tr_box = {}

                    def tr_half(lo, st=st, box=tr_box):
                        if lo == 0:
                            box["aux"] = ps_aux.tile([128, 512], F32,
                                                     tag="aux", name="tr")
                        tr = box["aux"].bitcast(BF16)
                        for t in range(lo, lo + 4):
                            nc.tensor.transpose(tr[:, 128 * t:128 * (t + 1)],
                                                st[:, 128 * t:128 * (t + 1)],
                                                ident[:])

                    def tr_copy(p=p, qc=qc, box=tr_box):
                        nc.vector.tensor_copy(
                            plhsT[p][:, qc * 1024:(qc + 1) * 1024],
                            box["aux"].bitcast(BF16))

                    pv_queue.append((None, lambda: tr_half(0)))
                    pv_queue.append((None, lambda: tr_half(4)))
                    pv_queue.append((None, tr_copy))
                if inline_tr:
                    pump_pv(len(pv_queue))

            st_seg = {}

            # ---- 512-wide half segment: used to split the very last
            # (head, q-chunk) so the final projection tiles overlap the
            # second half's softmax stream
            def half_segment(h, qc, qh, fillers):
                p, hl = h // 2, h % 2
                pv = ps_pv.tile([128, 1024], F32, tag="pv", name="pv")
                if hl == 0 and qh == 0:
                    st_seg[p] = st_pool.tile([128, 1024], BF16, tag="st",
                                             name="st")
                st = st_seg[p]
                q0 = qc * 1024 + qh * 512

                def scores(kt):
                    sc = ps_sc.tile([128, 512], F32,
                                    tag=("sca", "scb")[kt % 2], name="sc")
                    ksl = slice(kt * 128, (kt + 1) * 128)
                    nc.tensor.matmul(
                        sc[:],
                        kt_sb[p][64 * hl:64 * hl + 64, ksl],
                        qt_sb[p][64 * hl:64 * hl + 64, q0:q0 + 512],
                        start=True, stop=True)
                    return sc

                scs = [scores(0), scores(1)]
                for kt in range(KT):
                    pt = pt_pool.tile([128, 512], BF16, tag="pth", name="pt",
                                      bufs=6)
                    if kt in FAST_KT:
                        nc.vector.tensor_scalar(pt[:].bitcast(I16),
                                                scs[kt % 2][:],
                                                FE_A, FE_B, AluMult, AluAdd)
                    else:
                        nc.scalar.activation(pt[:],
                                             scs[kt % 2][:],
                                             Exp, scale=SCALE)
                    if fillers:
                        fillers.pop(0)()
                    if kt + 2 < KT:
                        scs[kt % 2] = scores(kt + 2)
                    for t in range(4):
                        nc.tensor.matmul(
                            pv[:, 65 * t:65 * t + 65],
                            pt[:, 128 * t:128 * (t + 1)],
                            va_sb[kt][:, h * 65:(h + 1) * 65],
                            start=(kt == 0 and t == 0),
                            stop=(kt == KT - 1),
                            skip_group_check=True)

                recip = rc_pool.tile([128, 8], F32, tag="rc", name="recip")
                nc.vector.reciprocal(recip[:, 0:4], pv[:, 64:260:65])
                st3 = st[:].rearrange("p (t d) -> p t d", d=128)[
                    :, 4 * qh:4 * qh + 4, 64 * hl:64 * hl + 64]
                pv3 = pv[:, 0:260].rearrange("p (t d) -> p t d", d=65)[
                    :, :, 0:64]
                rc3 = recip[:, 0:4].rearrange("p (t d) -> p t d", d=1)
                nc.vector.tensor_mul(st3, pv3, rc3.broadcast_to((128, 4, 64)))
                if hl == 1:
                    aux = ps_aux.tile([128, 512], F32, tag="aux", name="tr")
                    tr = aux.bitcast(BF16)
                    for t in range(4):
                        nc.tensor.transpose(
                            tr[:, 128 * t:128 * (t + 1)],
                            st[:, 128 * (4 * qh + t):128 * (4 * qh + t + 1)],
                            ident[:])
                    nc.vector.tensor_copy(
                        plhsT[p][:, q0:q0 + 512], tr[:, 0:512])

            # ---------------- emission schedule ----------------
            # p-state warmup: the PE ramps to full clock only after ~3us of
            # continuous busy; fill the initial DMA-wait gap with dummy
            # matmuls on a memset tile so real matmuls start at full speed
            warm = pp.tile([128, 128], BF16, tag="warm")
            nc.vector.memset(warm[:], 0.0)
            wacc = ps_aux.tile([128, 512], F32, tag="aux", name="warm_acc")
            for _ in range(40):
                nc.tensor.matmul(wacc[:, 0:128], warm[:], warm[:],
                                 start=True, stop=True)
            # head: only what the first exps need (V rows 0-1, pair 0's
            # low-q Q packs and first K chunk); everything else fills
            # segment slots just-in-time
            v_tile(0)
            v_tile(1)
            v_tile(2)
            v_tile(3)
            v_tile(4)
            if USE_FP8:
                for i in (0, 1, 4):
                    qk_chunk(0, i)
            else:
                qk_chunk2(0, 0, 1)
                qk_chunk(0, 4)
                # pair-1 low-q chunks also consume (wqk, xt) arrivals: more
                # ready PE work per load during the DMA-bound cold start
                qk_chunk2(1, 0, 1)

            def pack_units(p, chunks):
                return [lambda p=p, i=i, half=half: qk_chunk_half(p, i, half)
                        for i in chunks for half in range(2)]

            def v_units(rts):
                return [lambda rt=rt, half=half: v_tile_half(rt, half)
                        for rt in rts for half in range(2)]

            # per-segment fillers, sized ~0.5-1 exp each, due before use:
            # seg 0 runs at 2 pops/kt to finish the V rows just-in-time;
            # pair p's K + low-q Q packs before seg 2p; high-q Q (needed by
            # qc1 only) in segs 5-8; projection of qc0 rows spread over qc1
            seg_fill = {i: [] for i in range(2 * HPC)}
            seg_fill[0] = (v_units([5]) + pack_units(0, [5])
                           + v_units([6, 7]) + pack_units(0, [6])
                           + v_units([8, 9]) + pack_units(0, [7])
                           + v_units([10, 11, 12, 13, 14, 15]))
            seg_fill[1] = pack_units(1, [0, 1, 4])
            seg_fill[2] = pack_units(1, [5, 6, 7]) + pack_units(2, [0])
            seg_fill[3] = pack_units(2, [1, 4]) + pack_units(3, [0])
            seg_fill[4] = pack_units(2, [5, 6, 7]) + pack_units(3, [1])
            seg_fill[5] = pack_units(3, [4]) + pack_units(0, [2, 3])
            seg_fill[6] = pack_units(3, [5, 6, 7])
            seg_fill[7] = pack_units(1, [2, 3])
            seg_fill[8] = pack_units(2, [2, 3])
            seg_fill[9] = pack_units(3, [2, 3])
            for s in range(HPC):
                seg_fill[8 + s] += [
                    lambda qt_i=s, nch=j: proj_half(qt_i, nch)
                    for j in range(2)]

            carry = []
            for qc in range(QC):
                for h in range(HPC):
                    if qc == 1 and h == HPC - 1:
                        break
                    fl = carry + seg_fill[qc * HPC + h]
                    segment(h, qc, fl,
                            budget=2,
                            inline_tr=(qc == 0 and h == HPC - 1))
                    carry = fl
            pump_pv(len(pv_queue))
            half_segment(HPC - 1, 1, 0, carry + seg_fill[15])
            tail_fill = [lambda qt_i=8 + (j // 2), nch=j % 2:
                         proj_half(qt_i, nch) for j in range(8)]
            half_segment(HPC - 1, 1, 1, tail_fill)
            for w in pending_pe:
                w()
            pending_pe.clear()
            for qt_i in range(12, 16):
                proj_half(qt_i, 0)
                proj_half(qt_i, 1)

    nc.compile()
    return nc


def _get_nc(with_bias=False):
    if with_bias not in _COMPILED:
        _COMPILED[with_bias] = _build(with_bias)
    return _COMPILED[with_bias]


def _prep_in_maps(x, W_qkv, b_qkv, W_proj, b_proj, with_bias):
    ident = np.eye(128).astype(bf)
    in_maps = []
    for c in range(N_CORES):
        b = c // 2
        g = c % 2
        hs = slice(g * 512, (g + 1) * 512)
        xt = np.ascontiguousarray(x[b].T).astype(bf)
        wq = W_qkv[:, 0:C][:, hs]
        wk = W_qkv[:, C:2 * C][:, hs]
        wv = W_qkv[:, 2 * C:3 * C][:, hs]
        wqk = np.ascontiguousarray(np.concatenate([wq, wk], axis=1)).astype(bf)
        wpr = np.ascontiguousarray(W_proj[hs, :]).astype(bf)
        m = {
            "xt": xt, "wqk": wqk, "wv": np.ascontiguousarray(wv).astype(bf),
            "wpr": wpr, "ident": ident,
        }
        if with_bias:
            bq = b_qkv[0:C][hs]
            bk = b_qkv[C:2 * C][hs]
            bvv = b_qkv[2 * C:3 * C][hs]
            m["bqk"] = np.concatenate([bq, bk])[None, :].astype(bf)
            m["bv"] = np.ascontiguousarray(bvv[None, :]).astype(bf)
            m["bpr"] = ((b_proj if g == 0 else np.zeros_like(b_proj))
                        [None, :].astype(bf))
        in_maps.append(m)
    return in_maps


def kernel(x, W_qkv, b_qkv, W_proj, b_proj):
    x = np.asarray(x, dtype=np.float32)
    W_qkv = np.asarray(W_qkv, dtype=np.float32)
    b_qkv = np.asarray(b_qkv, dtype=np.float32)
    W_proj = np.asarray(W_proj, dtype=np.float32)
    b_proj = np.asarray(b_proj, dtype=np.float32)
    with_bias = bool(np.any(b_qkv) or np.any(b_proj))
    nc = _get_nc(with_bias)
    in_maps = _prep_in_maps(x, W_qkv, b_qkv, W_proj, b_proj, with_bias)
    res = run_bass_kernel_spmd(nc, in_maps, core_ids=list(range(N_CORES)))
    out = np.empty((B, N, C), dtype=np.float32)
    for b in range(B):
        out[b] = (res.results[2 * b]["out"].astype(np.float32)
                  + res.results[2 * b + 1]["out"].astype(np.float32))
    return out



# revision 54
# speedup vs baseline: 1.1074x; 1.0050x over previous
"""Trainium2 Bass kernel for multi-head self-attention (B=4, N=2048, C=1024, H=16).

Sharding: 8 cores = 4 batches x 2 head-groups (8 heads each). Per core:
  - V rows and Q^T/K^T packs from x[b] (PE), V interleaved with a per-head
    ones column so PV also produces softmax row sums
  - flash-style attention per (head, 1024-q chunk): S^T tiles -> exp ->
    PV with out layout [q, d+1] (full 128 output partitions)
  - softmax exp is split across TWO engines per key-tile: ACT does an exact
    Exp on the low 512 q columns, DVE a Schraudolph fast-exp (int16 round of
    s*FE_A+FE_B bitcast to bf16, ~1.8% RMS) on the high 512. Each half reads
    its own single-reader PSUM scores tile, so the write-after-read for the
    kt+2 scores matmul clears after one 512-wide exp (~650ns) instead of a
    whole-tile serial chain; neither exp engine ever paces the PE backbone.
  - normalize by 1/rowsum via per-partition tensor_scalar_mul, PE-transpose
    back to [d, q] for the output projection
Host preps per-core inputs and sums the two partial projections per batch.
rel_err ~1.1e-2 (fast-exp on half the softmax weights), gate is 2e-2.
"""

import numpy as np
import ml_dtypes

import concourse.bass as bass
import concourse.mybir as mybir
import concourse.tile as tile
from concourse import bacc
from concourse.ap import AP
from concourse.bass_utils import run_bass_kernel_spmd

BF16 = mybir.dt.bfloat16
F32 = mybir.dt.float32
FP8 = mybir.dt.float8e4
I16 = mybir.dt.int16
Exp = mybir.ActivationFunctionType.Exp
DR = mybir.MatmulPerfMode.DoubleRow
AluMult = mybir.AluOpType.mult
AluAdd = mybir.AluOpType.add
bf = ml_dtypes.bfloat16

B, N, C = 4, 2048, 1024
H, D = 16, 64
N_CORES = 8
HPC = H // 2     # heads per core (8)
PAIRS = HPC // 2  # head pairs per core (4)
CT = C // 128    # contraction tiles over C (8)
KT = N // 128    # key tiles (16)
RT = N // 128    # row tiles for V (16)
QT4 = N // 512   # 512-wide q chunks (4)
QC = 2           # 1024-wide q chunks
SCALE = 1.0 / float(np.sqrt(D))
# Schraudolph fast-exp in bf16-bit space: exp(s*SCALE) ~= bitcast_bf16(
# int16(round(s * FE_A + FE_B))). FE_B tuned for min RMS rel err (~1.8%).
FE_A = 128.0 * 1.4426950408889634 * SCALE
FE_B = 16248.5
# kt indices whose exp runs as DVE fast-exp in the 512-wide half
# segments (rest on ACT)
FAST_KT = frozenset((1, 3, 5, 7, 9, 11, 13, 15))

import os
USE_FP8 = os.environ.get("K_FP8", "0") == "1"

_COMPILED = {}


def _build(with_bias: bool):
    nc = bacc.Bacc("TRN2", target_bir_lowering=False, debug=False,
                   num_devices=N_CORES)
    xt_d = nc.dram_tensor("xt", [C, N], BF16, kind="ExternalInput").ap()
    wqk_d = nc.dram_tensor("wqk", [C, 1024], BF16, kind="ExternalInput").ap()
    wv_d = nc.dram_tensor("wv", [C, 512], BF16, kind="ExternalInput").ap()
    wpr_d = nc.dram_tensor("wpr", [512, C], BF16, kind="ExternalInput").ap()
    id_d = nc.dram_tensor("ident", [128, 128], BF16, kind="ExternalInput").ap()
    if with_bias:
        bqk_d = nc.dram_tensor("bqk", [1, 1024], BF16, kind="ExternalInput").ap()
        bv_d = nc.dram_tensor("bv", [1, 512], BF16, kind="ExternalInput").ap()
        bpr_d = nc.dram_tensor("bpr", [1, C], BF16, kind="ExternalInput").ap()
    out_d = nc.dram_tensor("out", [N, C], BF16, kind="ExternalOutput").ap()

    with tile.TileContext(nc) as tc:
        with (
            tc.tile_pool(name="persist", bufs=1) as pp,
            tc.tile_pool(name="pt", bufs=24) as pt_pool,
            tc.tile_pool(name="st", bufs=3) as st_pool,
            tc.tile_pool(name="rc", bufs=6) as rc_pool,
            tc.tile_pool(name="q8s", bufs=4) as q8s_pool,
            tc.tile_pool(name="oc", bufs=4) as oc_pool,
            tc.tile_pool(name="ps_sc", bufs=2, space="PSUM") as ps_sc,
            tc.tile_pool(name="ps_pv", bufs=1, space="PSUM") as ps_pv,
            tc.tile_pool(name="ps_aux", bufs=2, space="PSUM") as ps_aux,
        ):
            # ---------------- input loads ----------------
            # alternate between the SP/HWDGE and Pool/SWDGE descriptor-gen
            # pipelines so bursts of DMAs generate in parallel
            dma_rr = [0]
            # first four loads (wv0,xt0,wv1,xt1) feed the very first PE
            # matmuls: split them across both descriptor-gen queues so the
            # first pair lands ~1us earlier than a single serial queue
            _head_eng = [nc.gpsimd, nc.sync, nc.sync, nc.gpsimd]

            def dma(dst, srcap):
                # HWDGE gen is ~625ns/DMA vs SWDGE ~1040ns: weight 2:1
                if dma_rr[0] < len(_head_eng):
                    eng = _head_eng[dma_rr[0]]
                else:
                    eng = (nc.sync, nc.sync, nc.gpsimd)[dma_rr[0] % 3]
                dma_rr[0] += 1
                eng.dma_start(dst, srcap)

            xt_sb = [pp.tile([128, N], BF16, tag=f"xt{ct}", name=f"xt{ct}")
                     for ct in range(CT)]
            wv_sb = [pp.tile([128, 512], BF16, tag=f"wv{ct}", name=f"wv{ct}")
                     for ct in range(CT)]
            wqk_sb = [pp.tile([128, 1024], BF16, tag=f"wqk{ct}", name=f"wqk{ct}")
                      for ct in range(CT)]
            for ct in range(CT):
                dma(wv_sb[ct][:], wv_d[ct * 128:(ct + 1) * 128, :])
                dma(xt_sb[ct][:, 0:512], xt_d[ct * 128:(ct + 1) * 128, 0:512])
            # wqk interleaved with the q4=1 x columns: the head's Q/K pack
            # chunks consume exactly (wqk[ct], xt[ct][512:1024]) per ct step
            for ct in range(CT):
                dma(wqk_sb[ct][:], wqk_d[ct * 128:(ct + 1) * 128, :])
                dma(xt_sb[ct][:, 512:1024],
                    xt_d[ct * 128:(ct + 1) * 128, 512:1024])
            for q4 in range(2, QT4):
                qsl = slice(q4 * 512, (q4 + 1) * 512)
                for ct in range(CT):
                    dma(xt_sb[ct][:, qsl], xt_d[ct * 128:(ct + 1) * 128, qsl])
            wpr_sb = []
            for cp in range(PAIRS):
                t = pp.tile([128, C], BF16, tag=f"wpr{cp}", name=f"wpr{cp}")
                dma(t[:], wpr_d[cp * 128:(cp + 1) * 128, :])
                wpr_sb.append(t)
            ident = pp.tile([128, 128], BF16, tag="ident")
            dma(ident[:], id_d[:])

            def xt_s(ct, sl):
                return xt_sb[ct][:, sl]

            def wqk_s(ct, sl):
                return wqk_sb[ct][:, sl]

            def wv_s(ct):
                return wv_sb[ct][:]

            def wpr_s(cp, sl):
                return wpr_sb[cp][:, sl]
            if with_bias:
                ones = pp.tile([1, N], BF16, tag="ones")
                nc.vector.memset(ones[:], 1.0)
                bqk_sb = pp.tile([1, 1024], BF16, tag="bqk")
                dma(bqk_sb[:], bqk_d[:])
                bv_sb = pp.tile([1, 512], BF16, tag="bv")
                dma(bv_sb[:], bv_d[:])
                bpr_sb = pp.tile([1, C], BF16, tag="bpr")
                dma(bpr_sb[:], bpr_d[:])

            # ---------------- persistent working tiles ----------------
            if USE_FP8:
                # fp8 Q/K packs: tile p holds the pair's 2 heads at partition
                # bases 0/64 (base 96 is not addressable by matmul operands);
                # per head layout [32, 2(d-half), N]
                q8_sb = [pp.tile([128, 2 * N], FP8, tag=f"q8{p}", name=f"q8{p}")
                         for p in range(PAIRS)]
                k8_sb = [pp.tile([128, 2 * N], FP8, tag=f"k8{p}", name=f"k8{p}")
                         for p in range(PAIRS)]
            else:
                qt_sb = [pp.tile([128, N], BF16, tag=f"qt{p}", name=f"qt{p}")
                         for p in range(PAIRS)]
                kt_sb = [pp.tile([128, N], BF16, tag=f"kt{p}", name=f"kt{p}")
                         for p in range(PAIRS)]
            va_sb = [pp.tile([128, HPC * 65], BF16, tag=f"va{rt}", name=f"va{rt}")
                     for rt in range(RT)]
            plhsT = [pp.tile([128, N], BF16, tag=f"pl{p}", name=f"pl{p}")
                     for p in range(PAIRS)]

            # ---- V row tile: out[r, h*65+d] = sum_c xT[c, r] * Wv[c, h*64+d]
            v_acc = {}

            def v_tile_half(rt, half):
                va3 = va_sb[rt][:].rearrange("p (h d) -> p h d", d=65)
                rsl = slice(rt * 128, (rt + 1) * 128)
                if half == 0:
                    nc.vector.memset(va3[:, :, 64:65], 1.0)
                    v_acc[rt] = ps_aux.tile([128, 512], F32, tag="aux",
                                            name="acc_v")
                acc = v_acc[rt]
                for ct in range(4 * half, 4 * half + 4):
                    nc.tensor.matmul(acc[:], xt_s(ct, rsl), wv_s(ct),
                                     start=(ct == 0),
                                     stop=(not with_bias and ct == CT - 1))
                if half == 0:
                    return
                if with_bias:
                    nc.tensor.matmul(acc[:], ones[0:1, 0:128], bv_sb[:],
                                     start=False, stop=True)
                src3 = acc[:].rearrange("p (h d) -> p h d", d=64)
                nc.vector.tensor_copy(va3[:, :, 0:64], src3)
                del v_acc[rt]

            def v_tile(rt):
                v_tile_half(rt, 0)
                v_tile_half(rt, 1)

            # ---- Q^T/K^T pack chunk: out[o, q] = sum_c W[c, o] * xT[c, q]
            # split into two PE half-units so fillers stay fine-grained
            qk_acc = {}

            def qk_chunk_half(p, i, half):
                qk, q4 = i // QT4, i % QT4
                osl = slice(qk * 512 + p * 128, qk * 512 + (p + 1) * 128)
                qsl = slice(q4 * 512, (q4 + 1) * 512)
                if half == 0:
                    qk_acc[(p, i)] = ps_aux.tile([128, 512], F32, tag="aux",
                                                 name="acc_qk")
                acc = qk_acc[(p, i)]
                for ct in range(4 * half, 4 * half + 4):
                    nc.tensor.matmul(acc[:], wqk_s(ct, osl),
                                     xt_s(ct, qsl), start=(ct == 0),
                                     stop=(not with_bias and ct == CT - 1))
                if half == 0:
                    return
                if with_bias:
                    nc.tensor.matmul(acc[:], bqk_sb[0:1, osl], ones[0:1, qsl],
                                     start=False, stop=True)
                if USE_FP8:
                    stage = q8s_pool.tile([128, 512], FP8, tag="q8s",
                                          name="stage8")
                    nc.vector.tensor_copy(stage[:], acc[:])
                    dst = (q8_sb, k8_sb)[qk]
                    for hl in range(2):
                        for ih in range(2):
                            dma(dst[p][64 * hl:64 * hl + 32,
                                       ih * N + q4 * 512:ih * N + (q4 + 1) * 512],
                                stage[64 * hl + 32 * ih:64 * hl + 32 * ih + 32, :])
                else:
                    dst = (qt_sb, kt_sb)[qk]
                    nc.scalar.copy(dst[p][:, qsl], acc[:])
                del qk_acc[(p, i)]

            def qk_chunk(p, i):
                qk_chunk_half(p, i, 0)
                qk_chunk_half(p, i, 1)

            # two chunks with ct-interleaved matmuls: both chains advance as
            # each wqk tile lands (avoids head-of-line blocking on the
            # streaming weight DMAs)
            def qk_chunk2(p, ia, ib):
                accs = {}
                for i in (ia, ib):
                    accs[i] = ps_aux.tile([128, 512], F32, tag="aux",
                                          name="acc_qk")
                for ct in range(CT):
                    for i in (ia, ib):
                        qk, q4 = i // QT4, i % QT4
                        osl = slice(qk * 512 + p * 128,
                                    qk * 512 + (p + 1) * 128)
                        qsl = slice(q4 * 512, (q4 + 1) * 512)
                        nc.tensor.matmul(accs[i][:], wqk_s(ct, osl),
                                         xt_s(ct, qsl), start=(ct == 0),
                                         stop=(not with_bias and ct == CT - 1))
                for i in (ia, ib):
                    qk, q4 = i // QT4, i % QT4
                    osl = slice(qk * 512 + p * 128, qk * 512 + (p + 1) * 128)
                    qsl = slice(q4 * 512, (q4 + 1) * 512)
                    if with_bias:
                        nc.tensor.matmul(accs[i][:], bqk_sb[0:1, osl],
                                         ones[0:1, qsl], start=False, stop=True)
                    dst = (qt_sb, kt_sb)[qk]
                    nc.scalar.copy(dst[p][:, qsl], accs[i][:])

            # ---- partial output projection for one 128-row q tile (half) ----
            def proj_half(qt_i, nch):
                qsl = slice(qt_i * 128, (qt_i + 1) * 128)
                nsl = slice(nch * 512, (nch + 1) * 512)
                if nch == 0:
                    proj_oc[qt_i] = oc_pool.tile([128, 1024], BF16, tag="oc",
                                                 name="oc")
                oc = proj_oc[qt_i]
                acc = ps_aux.tile([128, 512], F32, tag="aux", name="acc_pr")
                for cp in range(PAIRS):
                    nc.tensor.matmul(acc[:], plhsT[cp][:, qsl],
                                     wpr_s(cp, nsl), start=(cp == 0),
                                     stop=(not with_bias and cp == PAIRS - 1))
                if with_bias:
                    nc.tensor.matmul(acc[:], ones[0:1, 0:128], bpr_sb[0:1, nsl],
                                     start=False, stop=True)
                if qt_i >= 12:
                    # tail-critical: put one half-copy on the idle ACT engine
                    # and store with a single descriptor-gen pass (the HWDGE
                    # generator is one serial device; two 512-wide stores
                    # would serialize their gens on the critical tail)
                    (nc.vector.tensor_copy if nch == 0
                     else nc.scalar.copy)(oc[:, nsl], acc[:])
                    if nch == 1:
                        nc.sync.dma_start(out_d[qsl, :], oc[:])
                else:
                    # alternate copy engine to keep ACT/DVE balanced
                    (nc.vector.tensor_copy if (qt_i * 2 + nch) % 2 == 0
                     else nc.scalar.copy)(oc[:, nsl], acc[:])
                    if nch == 1:
                        dma(out_d[qsl, :], oc[:])

            proj_oc = {}

            # ---- attention segment: one head, one 1024-wide q chunk ----
            # pv accum regions: q-tile t<7 at col 65*t (bank 0), t=7 at col
            # 512 (bank 1). start=True on the first matmul into each bank
            # marks the whole bank pending-zero, so every region's first
            # write lands fresh; stop=True only on the bank's last matmul.
            pending_pe = []
            # decoupled PV stream: (exp_idx, closure). The softmax stream
            # (scores+exp) runs ahead; PV/drain/transpose work is pumped into
            # PE's spare capacity, bounded by the pt pool depth.
            pv_queue = []
            exp_idx = [0]
            PT_LAG = 14
            PT_BUFS = 16

            def pump_pv(n):
                for _ in range(n):
                    if pv_queue:
                        pv_queue.pop(0)[1]()

            # idle-slot pump: drain PV work but keep >=2 entries queued so
            # PV emission lags exp by ~2 kts and never eats exp latency
            def pump_keep(n, keep=3):
                for _ in range(n):
                    if len(pv_queue) > keep:
                        pv_queue.pop(0)[1]()

            def segment(h, qc, fillers, budget=1, inline_tr=False):
                p, hl = h // 2, h % 2
                g, r = h // 4, h % 4
                pv = ps_pv.tile([128, 1024], F32, tag="pv", name="pv")
                if hl == 0:
                    st_seg[p] = st_pool.tile([128, 1024], BF16, tag="st",
                                             name="st")
                st = st_seg[p]

                def pv_region(t):
                    c0 = 65 * t if t < 7 else 512
                    return pv[:, c0:c0 + 65]

                def scores(kt):
                    # two separate PSUM tiles (one per 512-wide matmul):
                    # PSUM tile deps are tile-granular, so separate tiles
                    # keep the ACT and DVE exp readers fully decoupled
                    ksl = slice(kt * 128, (kt + 1) * 128)
                    out = []
                    for qh in range(2):
                        q0 = qc * 1024 + qh * 512
                        sc = ps_sc.tile([128, 512], F32,
                                        tag=("sca", "scb")[qh], name="sc")
                        nc.tensor.matmul(
                            sc[:],
                            kt_sb[p][64 * hl:64 * hl + 64, ksl],
                            qt_sb[p][64 * hl:64 * hl + 64, q0:q0 + 512],
                            start=True, stop=True)
                        out.append(sc)
                    return out

                def pv_group(kt, pt, ptb):
                    for t in range(8):
                        lhsT = (pt[:, 128 * t:128 * (t + 1)] if t < 4
                                else ptb[:, 128 * (t - 4):128 * (t - 3)])
                        nc.tensor.matmul(
                            pv_region(t), lhsT,
                            va_sb[kt][:, h * 65:(h + 1) * 65],
                            start=(kt == 0 and t in (0, 7)),
                            stop=(kt == KT - 1),
                            skip_group_check=True)

                scs = [scores(0), scores(1)]
                for kt in range(KT):
                    # pt-slot WAR: the PV group reading the tile this exp's
                    # slot will recycle must be emitted before the exp
                    while pv_queue and (pv_queue[0][0] is None or
                                        pv_queue[0][0] <= exp_idx[0] - PT_LAG):
                        pump_pv(1)
                    # exp split by half across both engines: ACT does an
                    # exact Exp on sc_a, DVE a Schraudolph fast-exp on sc_b
                    # (int16 round of s*FE_A+FE_B, bitcast to bf16). Each
                    # half's sc tile has exactly one reader, so the WAR for
                    # the kt+2 scores matmul waits only its own 512-wide exp
                    pt = pt_pool.tile([128, 512], BF16, tag="pt", name="pt",
                                      bufs=PT_BUFS)
                    ptb = pt_pool.tile([128, 512], BF16, tag="ptb",
                                       name="ptb", bufs=PT_BUFS)
                    sca, scb = scs[kt % 2]
                    # fillers first: their ACT/DVE copies enqueue ahead of
                    # this kt's exps (filler kts have ample chain slack), so
                    # pack tiles land an exp earlier for the segments that
                    # consume them; their PE matmuls also ride out the exp
                    # latency before the WAR-gated lookahead scores
                    if kt < 6:
                        if fillers:
                            for _ in range(budget):
                                if fillers:
                                    fillers.pop(0)()
                        elif pending_pe:
                            pending_pe.pop(0)()
                        else:
                            pump_keep(2)
                    else:
                        if pending_pe:
                            pending_pe.pop(0)()
                        elif fillers:
                            for _ in range(budget):
                                if fillers:
                                    fillers.pop(0)()
                        else:
                            pump_keep(2)
                    nc.scalar.activation(pt[:], sca[:], Exp, scale=SCALE)
                    nc.vector.tensor_scalar(
                        ptb[:].bitcast(I16), scb[:],
                        FE_A, FE_B, AluMult, AluAdd)
                    if kt + 2 < KT:
                        scs[kt % 2] = scores(kt + 2)
                    pv_queue.append((exp_idx[0], lambda kt=kt, pt=pt, ptb=ptb:
                                     pv_group(kt, pt, ptb)))
                    exp_idx[0] += 1

                # normalize: recip of row sums, per-partition scalar multiply
                def drain(pv=pv, st=st, hl=hl):
                    recip = rc_pool.tile([128, 8], F32, tag="rc", name="recip")
                    nc.vector.reciprocal(recip[:, 0:7], pv[:, 64:455:65])
                    st3 = st[:].rearrange("p (t d) -> p t d", d=128)[
                        :, 0:7, 64 * hl:64 * hl + 64]
                    pv3 = pv[:, 0:455].rearrange("p (t d) -> p t d", d=65)[
                        :, :, 0:64]
                    rc3 = recip[:, 0:7].rearrange("p (t d) -> p t d", d=1)
                    nc.vector.tensor_mul(st3, pv3,
                                         rc3.broadcast_to((128, 7, 64)))
                    nc.vector.reciprocal(recip[:, 7:8], pv[:, 576:577])
                    nc.vector.tensor_scalar_mul(
                        st[:, 128 * 7 + 64 * hl:128 * 7 + 64 * hl + 64],
                        pv[:, 512:576], recip[:, 7:8])

                pv_queue.append((None, drain))
                if hl == 1:
                    # transpose [q, d-pair] -> [d-pair, q] via PE, then one
                    # copy into the proj lhsT pack; deferred into the next
                    # segment's filler slots so the PE stream never stalls
                    # on the DVE normalize chain
                    tr_box = {}

                    def tr_half(lo, st=st, box=tr_box):
                        if lo == 0:
                            box["aux"] = ps_aux.tile([128, 512], F32,
                                                     tag="aux", name="tr")
                        tr = box["aux"].bitcast(BF16)
                        for t in range(lo, lo + 4):
                            nc.tensor.transpose(tr[:, 128 * t:128 * (t + 1)],
                                                st[:, 128 * t:128 * (t + 1)],
                                                ident[:])

                    def tr_copy(p=p, qc=qc, box=tr_box):
                        nc.vector.tensor_copy(
                            plhsT[p][:, qc * 1024:(qc + 1) * 1024],
                            box["aux"].bitcast(BF16))

                    pv_queue.append((None, lambda: tr_half(0)))
                    pv_queue.append((None, lambda: tr_half(4)))
                    pv_queue.append((None, tr_copy))
                if inline_tr:
                    pump_pv(len(pv_queue))

            st_seg = {}

            # ---- 512-wide half segment: used to split the very last
            # (head, q-chunk) so the final projection tiles overlap the
            # second half's softmax stream
            def half_segment(h, qc, qh, fillers):
                p, hl = h // 2, h % 2
                pv = ps_pv.tile([128, 1024], F32, tag="pv", name="pv")
                if hl == 0 and qh == 0:
                    st_seg[p] = st_pool.tile([128, 1024], BF16, tag="st",
                                             name="st")
                st = st_seg[p]
                q0 = qc * 1024 + qh * 512

                def scores(kt):
                    sc = ps_sc.tile([128, 512], F32,
                                    tag=("sca", "scb")[kt % 2], name="sc")
                    ksl = slice(kt * 128, (kt + 1) * 128)
                    nc.tensor.matmul(
                        sc[:],
                        kt_sb[p][64 * hl:64 * hl + 64, ksl],
                        qt_sb[p][64 * hl:64 * hl + 64, q0:q0 + 512],
                        start=True, stop=True)
                    return sc

                scs = [scores(0), scores(1)]
                for kt in range(KT):
                    pt = pt_pool.tile([128, 512], BF16, tag="pth", name="pt",
                                      bufs=6)
                    if kt in FAST_KT:
                        nc.vector.tensor_scalar(pt[:].bitcast(I16),
                                                scs[kt % 2][:],
                                                FE_A, FE_B, AluMult, AluAdd)
                    else:
                        nc.scalar.activation(pt[:],
                                             scs[kt % 2][:],
                                             Exp, scale=SCALE)
                    if fillers:
                        fillers.pop(0)()
                    if kt + 2 < KT:
                        scs[kt % 2] = scores(kt + 2)
                    for t in range(4):
                        nc.tensor.matmul(
                            pv[:, 65 * t:65 * t + 65],
                            pt[:, 128 * t:128 * (t + 1)],
                            va_sb[kt][:, h * 65:(h + 1) * 65],
                            start=(kt == 0 and t == 0),
                            stop=(kt == KT - 1),
                            skip_group_check=True)

                recip = rc_pool.tile([128, 8], F32, tag="rc", name="recip")
                nc.vector.reciprocal(recip[:, 0:4], pv[:, 64:260:65])
                st3 = st[:].rearrange("p (t d) -> p t d", d=128)[
                    :, 4 * qh:4 * qh + 4, 64 * hl:64 * hl + 64]
                pv3 = pv[:, 0:260].rearrange("p (t d) -> p t d", d=65)[
                    :, :, 0:64]
                rc3 = recip[:, 0:4].rearrange("p (t d) -> p t d", d=1)
                nc.vector.tensor_mul(st3, pv3, rc3.broadcast_to((128, 4, 64)))
                if hl == 1:
                    aux = ps_aux.tile([128, 512], F32, tag="aux", name="tr")
                    tr = aux.bitcast(BF16)
                    for t in range(4):
                        nc.tensor.transpose(
                            tr[:, 128 * t:128 * (t + 1)],
                            st[:, 128 * (4 * qh + t):128 * (4 * qh + t + 1)],
                            ident[:])
                    nc.vector.tensor_copy(
                        plhsT[p][:, q0:q0 + 512], tr[:, 0:512])

            # ---------------- emission schedule ----------------
            # p-state warmup: the PE ramps to full clock only after ~3us of
            # continuous busy; fill the initial DMA-wait gap with dummy
            # matmuls on a memset tile so real matmuls start at full speed
            warm = pp.tile([128, 128], BF16, tag="warm")
            nc.vector.memset(warm[:], 0.0)
            wacc = ps_aux.tile([128, 512], F32, tag="aux", name="warm_acc")
            for _ in range(40):
                nc.tensor.matmul(wacc[:, 0:128], warm[:], warm[:],
                                 start=True, stop=True)
            # head: only what the first exps need (V rows 0-1, pair 0's
            # low-q Q packs and first K chunk); everything else fills
            # segment slots just-in-time
            v_tile(0)
            v_tile(1)
            v_tile(2)
            v_tile(3)
            v_tile(4)
            if USE_FP8:
                for i in (0, 1, 4):
                    qk_chunk(0, i)
            else:
                qk_chunk2(0, 0, 1)
                qk_chunk(0, 4)
                # pair-1 low-q chunks also consume (wqk, xt) arrivals: more
                # ready PE work per load during the DMA-bound cold start
                qk_chunk2(1, 0, 1)

            def pack_units(p, chunks):
                return [lambda p=p, i=i, half=half: qk_chunk_half(p, i, half)
                        for i in chunks for half in range(2)]

            def v_units(rts):
                return [lambda rt=rt, half=half: v_tile_half(rt, half)
                        for rt in rts for half in range(2)]

            # per-segment fillers, sized ~0.5-1 exp each, due before use:
            # seg 0 runs at 2 pops/kt to finish the V rows just-in-time;
            # pair p's K + low-q Q packs before seg 2p; high-q Q (needed by
            # qc1 only) in segs 5-8; projection of qc0 rows spread over qc1
            seg_fill = {i: [] for i in range(2 * HPC)}
            seg_fill[0] = (v_units([5]) + pack_units(0, [5])
                           + v_units([6, 7]) + pack_units(0, [6])
                           + v_units([8, 9]) + pack_units(0, [7])
                           + v_units([10, 11, 12, 13, 14, 15]))
            seg_fill[1] = pack_units(1, [4])
            seg_fill[2] = pack_units(1, [5, 6, 7]) + pack_units(2, [0])
            seg_fill[3] = pack_units(2, [1, 4]) + pack_units(3, [0])
            seg_fill[4] = pack_units(2, [5, 6, 7]) + pack_units(3, [1])
            seg_fill[5] = pack_units(3, [4]) + pack_units(0, [2, 3])
            seg_fill[6] = pack_units(3, [5, 6, 7])
            seg_fill[7] = pack_units(1, [2, 3])
            seg_fill[8] = pack_units(2, [2, 3])
            seg_fill[9] = pack_units(3, [2, 3])
            for s in range(HPC):
                seg_fill[8 + s] += [
                    lambda qt_i=s, nch=j: proj_half(qt_i, nch)
                    for j in range(2)]

            carry = []
            for qc in range(QC):
                for h in range(HPC):
                    if qc == 1 and h == HPC - 1:
                        break
                    fl = carry + seg_fill[qc * HPC + h]
                    segment(h, qc, fl,
                            budget=2,
                            inline_tr=(qc == 0 and h == HPC - 1))
                    carry = fl
            pump_pv(len(pv_queue))
            half_segment(HPC - 1, 1, 0, carry + seg_fill[15])
            tail_fill = [lambda qt_i=8 + (j // 2), nch=j % 2:
                         proj_half(qt_i, nch) for j in range(8)]
            half_segment(HPC - 1, 1, 1, tail_fill)
            for w in pending_pe:
                w()
            pending_pe.clear()
            for qt_i in range(12, 16):
                proj_half(qt_i, 0)
                proj_half(qt_i, 1)

    nc.compile()
    return nc


def _get_nc(with_bias=False):
    if with_bias not in _COMPILED:
        _COMPILED[with_bias] = _build(with_bias)
    return _COMPILED[with_bias]


def _prep_in_maps(x, W_qkv, b_qkv, W_proj, b_proj, with_bias):
    ident = np.eye(128).astype(bf)
    in_maps = []
    for c in range(N_CORES):
        b = c // 2
        g = c % 2
        hs = slice(g * 512, (g + 1) * 512)
        xt = np.ascontiguousarray(x[b].T).astype(bf)
        wq = W_qkv[:, 0:C][:, hs]
        wk = W_qkv[:, C:2 * C][:, hs]
        wv = W_qkv[:, 2 * C:3 * C][:, hs]
        wqk = np.ascontiguousarray(np.concatenate([wq, wk], axis=1)).astype(bf)
        wpr = np.ascontiguousarray(W_proj[hs, :]).astype(bf)
        m = {
            "xt": xt, "wqk": wqk, "wv": np.ascontiguousarray(wv).astype(bf),
            "wpr": wpr, "ident": ident,
        }
        if with_bias:
            bq = b_qkv[0:C][hs]
            bk = b_qkv[C:2 * C][hs]
            bvv = b_qkv[2 * C:3 * C][hs]
            m["bqk"] = np.concatenate([bq, bk])[None, :].astype(bf)
            m["bv"] = np.ascontiguousarray(bvv[None, :]).astype(bf)
            m["bpr"] = ((b_proj if g == 0 else np.zeros_like(b_proj))
                        [None, :].astype(bf))
        in_maps.append(m)
    return in_maps


def kernel(x, W_qkv, b_qkv, W_proj, b_proj):
    x = np.asarray(x, dtype=np.float32)
    W_qkv = np.asarray(W_qkv, dtype=np.float32)
    b_qkv = np.asarray(b_qkv, dtype=np.float32)
    W_proj = np.asarray(W_proj, dtype=np.float32)
    b_proj = np.asarray(b_proj, dtype=np.float32)
    with_bias = bool(np.any(b_qkv) or np.any(b_proj))
    nc = _get_nc(with_bias)
    in_maps = _prep_in_maps(x, W_qkv, b_qkv, W_proj, b_proj, with_bias)
    res = run_bass_kernel_spmd(nc, in_maps, core_ids=list(range(N_CORES)))
    out = np.empty((B, N, C), dtype=np.float32)
    for b in range(B):
        out[b] = (res.results[2 * b]["out"].astype(np.float32)
                  + res.results[2 * b + 1]["out"].astype(np.float32))
    return out

